# revision 9
# baseline (speedup 1.0000x reference)
# Correlation2D (RAFT-style correlation pyramid lookup) on 8 TRN2 NeuronCores.
#
# Sharding: data-parallel over the bs*h*w query axis. Each core owns 1024
# queries (= 8 image rows). Per block of 128 queries it computes its slice of
# the cost volume via a bf16 GEMM (fmap2 replicated, pooling folded into
# fmap2), writes the 4-level pyramid per-query-contiguous to DRAM (bf16), and
# gathers ONE contiguous run per (block, level) spanning the whole 10x10
# patch (rows are Wl apart inside the run; the bilinear stage reads the run
# through a strided view). Bilinear combine is separable in bf16, done in two
# half-batches (blocks 0-3 / 4-7) so it overlaps the GEMM pipeline; output is
# PE-transposed to channel-major. Output per core is [324, 8, 128] f32, host
# concatenates along y.
import numpy as np

# ---- problem constants (hardcoded per contest contract) ----
H, W = 64, 128
D = 256
NUM_LEVELS = 4
RADIUS = 4
KK = 2 * RADIUS + 1        # 9
PS = KK + 1                # 10x10 patch per (query, level)
NCORES = 8
QPC = (H * W) // NCORES    # 1024 queries per core
NBLK = QPC // 128          # 8 blocks of 128 queries
LVL_W = [W >> l for l in range(NUM_LEVELS)]            # 128 64 32 16
LVL_H = [H >> l for l in range(NUM_LEVELS)]            # 64 32 16 8
LVL_N = [LVL_W[l] * LVL_H[l] for l in range(NUM_LEVELS)]   # 8192 2048 512 128
LVL_OFF = [sum(LVL_N[:l]) for l in range(NUM_LEVELS)]  # 0 8192 10240 10752
LVLSUM = sum(LVL_N)        # 10880
RUN = [KK * LVL_W[l] + PS for l in range(NUM_LEVELS)]  # 1162 586 298 154
ROWL = [PS * LVL_W[l] for l in range(NUM_LEVELS)]      # 1280 640 320 160
PAD = 1024                 # zeroed head/tail pad (elements) per block buffer
BQS = 128 * LVLSUM         # elements of cv per block
BTOT = PAD + BQS + PAD     # per-block DRAM tensor elements (bf16)
NCH = NUM_LEVELS * KK * KK  # 324 output channels
MM_N = 512                 # matmul N-chunk (one PSUM bank of f32)
CHUNKS = [(0, 128), (128, 128), (256, NCH - 256)]  # output channel chunks

_CACHE = {}


def _emit(ctx, tc, out_ext, f1c, f2, crd, dbg=None):
    import concourse.bass as bass
    import concourse.mybir as mybir
    from concourse.masks import make_identity

    nc = tc.nc
    f32 = mybir.dt.float32
    bf16 = mybir.dt.bfloat16
    i32 = mybir.dt.int32
    Alu = mybir.AluOpType

    const_pool = ctx.enter_context(tc.tile_pool(name="constp", bufs=1))
    f2_pool = ctx.enter_context(tc.tile_pool(name="f2p", bufs=1))
    f1_pool = ctx.enter_context(tc.tile_pool(name="f1p", bufs=1))
    coordp = ctx.enter_context(tc.tile_pool(name="coordp", bufs=1))
    small = ctx.enter_context(tc.tile_pool(name="small", bufs=2))
    cvp = ctx.enter_context(tc.tile_pool(name="cvp", bufs=2))
    patchp = ctx.enter_context(tc.tile_pool(name="patchp", bufs=1))
    txp = ctx.enter_context(tc.tile_pool(name="txp", bufs=2))
    outp = ctx.enter_context(tc.tile_pool(name="outp", bufs=1))
    psum = ctx.enter_context(tc.tile_pool(name="psum", bufs=6, space="PSUM"))
    psum_t = ctx.enter_context(tc.tile_pool(name="psumt", bufs=2, space="PSUM"))
    dramp = ctx.enter_context(tc.tile_pool(name="dramp", bufs=1, space="DRAM"))

    # ------------- per-block DRAM cv buffers (query-contiguous pyramid) ----
    cv_dram = [dramp.tile([BTOT], bf16, name=f"cv_dram{b}") for b in range(NBLK)]

    # ---------------- input loads (ordered: small/critical first) ----------
    cxs = coordp.tile([128, NBLK], f32, name="cxs")
    cys = coordp.tile([128, NBLK], f32, name="cys")
    nc.scalar.dma_start(out=cxs[:], in_=crd[0, :].rearrange("(b p) -> p b", p=128))
    nc.scalar.dma_start(out=cys[:], in_=crd[1, :].rearrange("(b p) -> p b", p=128))

    f1h = []
    for k in range(2):
        t = f1_pool.tile([128, QPC], bf16, name=f"f1h{k}")
        nc.sync.dma_start(out=t[:], in_=f1c[k * 128 : (k + 1) * 128, :])
        f1h.append(t)

    # f2 halves in 2 sub-loads each so the first GEMM chunks can start early
    f2_halves = []
    for k in range(2):
        f2h = f2_pool.tile([128, LVL_N[0]], bf16, name=f"f2h{k}")
        f2_halves.append(f2h)
    for n0 in (0, LVL_N[0] // 2):
        for k in range(2):
            nc.sync.dma_start(
                out=f2_halves[k][:, n0 : n0 + LVL_N[0] // 2],
                in_=f2[k * 128 : (k + 1) * 128, n0 : n0 + LVL_N[0] // 2],
            )

    # zero head/tail pads (gather runs can poke into them; must stay finite)
    ztile = const_pool.tile([128, 8], bf16, name="ztile")
    nc.vector.memset(ztile[:], 0.0)
    for b in range(NBLK):
        nc.sync.dma_start(
            out=cv_dram[b][0:PAD].rearrange("(p x) -> p x", p=128), in_=ztile[:]
        )
        nc.sync.dma_start(
            out=cv_dram[b][PAD + BQS : BTOT].rearrange("(p x) -> p x", p=128),
            in_=ztile[:],
        )

    # ---------------- constants -------------------------------------------
    # per-level vectors [128, 4] (exact values via memset per lane)
    invv = const_pool.tile([128, NUM_LEVELS], f32, name="invv")
    wlv = const_pool.tile([128, NUM_LEVELS], f32, name="wlv")
    limxv = const_pool.tile([128, NUM_LEVELS], f32, name="limxv")
    limyv = const_pool.tile([128, NUM_LEVELS], f32, name="limyv")
    cvecv = const_pool.tile([128, NUM_LEVELS], f32, name="cvecv")
    for l in range(NUM_LEVELS):
        nc.gpsimd.memset(invv[:, l : l + 1], 1.0 / (1 << l))
        nc.gpsimd.memset(wlv[:, l : l + 1], float(LVL_W[l]))
        nc.gpsimd.memset(limxv[:, l : l + 1], float(LVL_W[l] - 1))
        nc.gpsimd.memset(limyv[:, l : l + 1], float(LVL_H[l] - 1))
        nc.gpsimd.memset(
            cvecv[:, l : l + 1],
            float(PAD + LVL_OFF[l] - RADIUS * LVL_W[l] - RADIUS),
        )

    # c ramp: -4..5 (patch-col -> absolute offset from floor(coord))
    cramp_i = const_pool.tile([128, PS], i32, name="cramp_i")
    nc.gpsimd.iota(cramp_i[:], pattern=[[1, PS]], base=-RADIUS, channel_multiplier=0)
    crampf = const_pool.tile([128, PS], f32, name="crampf")
    nc.gpsimd.tensor_copy(out=crampf[:], in_=cramp_i[:])

    # per-query element base offset of its pyramid inside its block's buffer
    pq_i = coordp.tile([128, 1], i32, name="pq_i")
    nc.gpsimd.iota(pq_i[:], pattern=[[1, 1]], base=0, channel_multiplier=1)
    bqf = coordp.tile([128, 1], f32, name="bqf")
    nc.gpsimd.tensor_copy(out=bqf[:], in_=pq_i[:])
    nc.gpsimd.tensor_scalar_mul(bqf[:], bqf[:], float(LVLSUM))

    ident = const_pool.tile([128, 128], bf16, name="ident")
    make_identity(nc, ident[:])

    # weight / index tiles
    idx_i = coordp.tile([128, NBLK, NUM_LEVELS], i32, name="idx_i")
    wx0e = coordp.tile([128, NBLK, NUM_LEVELS, KK], bf16, name="wx0e")
    wx1e = coordp.tile([128, NBLK, NUM_LEVELS, KK], bf16, name="wx1e")
    wy0e = coordp.tile([128, NBLK, NUM_LEVELS, KK], bf16, name="wy0e")
    wy1e = coordp.tile([128, NBLK, NUM_LEVELS, KK], bf16, name="wy1e")

    def emit_weights_idx():
        """All 4 levels batched: [128, NBLK, 4] / [128, NBLK, 4, PS] ops."""
        sh3 = [128, NBLK, NUM_LEVELS]

        def floor_frac(src, nm):
            # xs = src / 2^l ; floor via i32 cast (rounds on HW) + is_gt fix
            xs = small.tile(sh3, f32, name=f"xs_{nm}", tag="xs")
            nc.vector.tensor_tensor(
                xs[:],
                src[:].unsqueeze(2).to_broadcast(sh3),
                invv[:].unsqueeze(1).to_broadcast(sh3),
                op=Alu.mult,
            )
            ii = small.tile(sh3, i32, name=f"ii_{nm}", tag="ii")
            nc.vector.tensor_copy(out=ii[:], in_=xs[:])
            ff = small.tile(sh3, f32, name=f"ff_{nm}", tag="ff")
            nc.vector.tensor_copy(out=ff[:], in_=ii[:])
            adj = small.tile(sh3, f32, name=f"adj_{nm}", tag="adj")
            nc.vector.tensor_tensor(adj[:], ff[:], xs[:], op=Alu.is_gt)
            nc.vector.tensor_tensor(ff[:], ff[:], adj[:], op=Alu.subtract)
            fr = small.tile(sh3, f32, name=f"fr_{nm}", tag="fr")
            nc.vector.tensor_tensor(fr[:], xs[:], ff[:], op=Alu.subtract)
            return ff, fr

        ixf, fx = floor_frac(cxs, "x")
        iyf, fy = floor_frac(cys, "y")

        sh4 = [128, NBLK, NUM_LEVELS, PS]
        for (w0t, w1t, frac, posf, limv) in (
            (wx0e, wx1e, fx, ixf, limxv),
            (wy0e, wy1e, fy, iyf, limyv),
        ):
            pos = small.tile(sh4, f32, name="pos", tag="pos")
            nc.vector.tensor_tensor(
                pos[:],
                posf[:].unsqueeze(3).to_broadcast(sh4),
                crampf[:].unsqueeze(1).unsqueeze(1).to_broadcast(sh4),
                op=Alu.add,
            )
            # in-bounds <=> |2*pos - lim| <= lim
            nc.vector.tensor_scalar_mul(pos[:], pos[:], 2.0)
            nc.vector.tensor_tensor(
                pos[:], pos[:],
                limv[:].unsqueeze(1).unsqueeze(3).to_broadcast(sh4),
                op=Alu.subtract,
            )
            ok = small.tile(sh4, f32, name="ok", tag="ok")
            nc.scalar.activation(ok[:], pos[:], mybir.ActivationFunctionType.Abs)
            nc.vector.tensor_tensor(
                ok[:], ok[:],
                limv[:].unsqueeze(1).unsqueeze(3).to_broadcast(sh4),
                op=Alu.is_le,
            )
            w0 = small.tile(sh3, f32, name="w0", tag="w0")
            nc.vector.tensor_scalar(w0[:], frac[:], -1.0, 1.0,
                                    op0=Alu.mult, op1=Alu.add)  # 1 - frac
            shk = [128, NBLK, NUM_LEVELS, KK]
            nc.vector.tensor_tensor(
                w0t[:], w0[:].unsqueeze(3).to_broadcast(shk),
                ok[:, :, :, 0:KK], op=Alu.mult,
            )
            nc.vector.tensor_tensor(
                w1t[:], frac[:].unsqueeze(3).to_broadcast(shk),
                ok[:, :, :, 1:PS], op=Alu.mult,
            )

        # gather run start: PAD + p*LVLSUM + lvl_off + (iy-4)*Wl + (ix-4)
        t1 = small.tile(sh3, f32, name="t1", tag="t1")
        nc.vector.tensor_tensor(
            t1[:], iyf[:], wlv[:].unsqueeze(1).to_broadcast(sh3), op=Alu.mult
        )
        nc.vector.tensor_tensor(t1[:], t1[:], ixf[:], op=Alu.add)
        nc.vector.tensor_tensor(
            t1[:], t1[:],
            bqf[:].unsqueeze(2).to_broadcast(sh3), op=Alu.add,
        )
        nc.vector.tensor_tensor(
            t1[:], t1[:], cvecv[:].unsqueeze(1).to_broadcast(sh3), op=Alu.add
        )
        nc.vector.tensor_copy(out=idx_i[:], in_=t1[:])  # exact ints

    # ---------------- fmap2 pyramid pooling --------------------------------
    # pooled levels keep raw SUMS; the 1/16 * 0.25^l scale is in the drain
    f2_lv = [f2_halves]
    for l in range(1, NUM_LEVELS):
        Wl, Hl = LVL_W[l], LVL_H[l]
        pw, ph = LVL_W[l - 1], LVL_H[l - 1]
        halves = []
        for k in range(2):
            prev = f2_lv[l - 1][k][:].rearrange(
                "p (h w two) -> p h w two", h=ph, w=pw // 2, two=2
            )
            s1 = small.tile(
                [128, ph, pw // 2], bf16, name=f"s1_{l}_{k}", tag="poolt", bufs=1
            )
            nc.vector.tensor_tensor(
                s1[:], prev[:, :, :, 0], prev[:, :, :, 1], op=Alu.add
            )
            s1v = s1[:].rearrange("p (h2 two) w -> p h2 two w", h2=Hl, two=2)
            cur = f2_pool.tile([128, Hl * Wl], bf16, name=f"f2l{l}_{k}")
            curv = cur[:].rearrange("p (h w) -> p h w", h=Hl, w=Wl)
            nc.vector.tensor_tensor(
                curv[:], s1v[:, :, 0, :], s1v[:, :, 1, :], op=Alu.add
            )
            halves.append(cur)
        f2_lv.append(halves)

    # ---------------- patch tiles (gather destinations) --------------------
    patch = [
        patchp.tile([128, NBLK, ROWL[l]], bf16, name=f"patch{l}")
        for l in range(NUM_LEVELS)
    ]
    outq = patchp.tile([128, NBLK, NUM_LEVELS, KK, KK], bf16, name="outq")
    outq_v = outq[:].rearrange("p b l dy dx -> p b (l dy dx)")
    outT = [
        outp.tile([128, NBLK, 128], f32, name=f"outT{k}")
        for k in range(len(CHUNKS))
    ]

    def emit_bilinear_half(h):
        """Bilinear + transpose for blocks [4h, 4h+4)."""
        b0, b1 = 4 * h, 4 * h + 4
        nb = b1 - b0
        for l in range(NUM_LEVELS):
            Wl = LVL_W[l]
            Pv = patch[l][:].rearrange("p b (r c) -> p b r c", r=PS, c=Wl)
            bshape_x = [128, nb, PS, KK]
            tx = txp.tile([128, nb, PS, KK], bf16, name=f"tx{h}{l}", tag="tx")
            tx2 = txp.tile([128, nb, PS, KK], bf16, name=f"tx2{h}{l}", tag="tx2")
            nc.vector.tensor_tensor(
                tx[:], Pv[:, b0:b1, :, 0:KK],
                wx0e[:, b0:b1, l, :].unsqueeze(2).to_broadcast(bshape_x),
                op=Alu.mult,
            )
            nc.vector.tensor_tensor(
                tx2[:], Pv[:, b0:b1, :, 1:PS],
                wx1e[:, b0:b1, l, :].unsqueeze(2).to_broadcast(bshape_x),
                op=Alu.mult,
            )
            nc.vector.tensor_tensor(tx[:], tx[:], tx2[:], op=Alu.add)

            bshape_y = [128, nb, KK, KK]
            oq2 = txp.tile([128, nb, KK, KK], bf16, name=f"oq2{h}{l}", tag="oq2")
            nc.vector.tensor_tensor(
                oq2[:], tx[:, :, 0:KK, :],
                wy0e[:, b0:b1, l, :].unsqueeze(3).to_broadcast(bshape_y),
                op=Alu.mult,
            )
            nc.vector.tensor_tensor(
                outq[:, b0:b1, l], tx[:, :, 1:PS, :],
                wy1e[:, b0:b1, l, :].unsqueeze(3).to_broadcast(bshape_y),
                op=Alu.mult,
            )
            nc.vector.tensor_tensor(
                outq[:, b0:b1, l], outq[:, b0:b1, l], oq2[:], op=Alu.add
            )
        # transpose to channel-major
        for k, (c0, nk) in enumerate(CHUNKS):
            for b in range(b0, b1):
                ptt = psum_t.tile([128, 128], bf16, name="ptt", tag="ptt")
                nc.tensor.transpose(
                    out=ptt[:nk, :], in_=outq_v[:, b, c0 : c0 + nk],
                    identity=ident[:],
                )
                if b % 2 == 0:
                    nc.vector.tensor_copy(out=outT[k][0:nk, b, :], in_=ptt[:nk, :])
                else:
                    nc.scalar.copy(out=outT[k][0:nk, b, :], in_=ptt[:nk, :])
        for k, (c0, nk) in enumerate(CHUNKS):
            nc.sync.dma_start(
                out=out_ext[c0 : c0 + nk, b0:b1, :], in_=outT[k][0:nk, b0:b1, :]
            )

    # ---------------- per-block GEMM -> cv write -> gather ------------------
    drain_parity = 0
    for b in range(NBLK):
        cv_sb = cvp.tile([128, LVLSUM], bf16, name="cv_sb", tag="cv_sb")
        for l in range(NUM_LEVELS):
            scale_l = (1.0 / 16.0) * (0.25 ** l)
            Nl = LVL_N[l]
            for n0 in range(0, Nl, MM_N):
                n1 = min(Nl, n0 + MM_N)
                pt = psum.tile([128, n1 - n0], f32, name="pt", tag="pt")
                nc.tensor.matmul(
                    pt[:],
                    f1h[0][:, b * 128 : (b + 1) * 128],
                    f2_lv[l][0][:, n0:n1],
                    start=True,
                    stop=False,
                )
                nc.tensor.matmul(
                    pt[:],
                    f1h[1][:, b * 128 : (b + 1) * 128],
                    f2_lv[l][1][:, n0:n1],
                    start=False,
                    stop=True,
                )
                dst = cv_sb[:, LVL_OFF[l] + n0 : LVL_OFF[l] + n1]
                if drain_parity % 2 == 0:
                    nc.vector.tensor_scalar_mul(dst[:], pt[:], scale_l)
                else:
                    nc.scalar.mul(dst[:], pt[:], scale_l)
                drain_parity += 1
        # one DMA for the whole block's cv (rows 21.76KB contiguous)
        nc.sync.dma_start(
            out=cv_dram[b][PAD : PAD + BQS].rearrange("(q s) -> q s", s=LVLSUM),
            in_=cv_sb[:],
        )
        if b == 0:
            # emitted here (not up front) so block 0's drains lead the vector
            # stream and the tensor engine isn't stalled on PSUM at startup
            emit_weights_idx()
        # one indirect gather per level: contiguous run covering the patch
        cv2d = cv_dram[b][:].rearrange("(a x) -> a x", a=1024)
        for l in range(NUM_LEVELS):
            nc.gpsimd.indirect_dma_start(
                out=patch[l][:, b, 0 : RUN[l]],
                out_offset=None,
                in_=cv2d,
                in_offset=bass.IndirectOffsetOnAxis(
                    ap=idx_i[:, b, l].unsqueeze(1), axis=1
                ),
            )
        if b == NBLK // 2 - 1:
            emit_bilinear_half(0)
    emit_bilinear_half(1)

    if dbg is not None:
        nc.sync.dma_start(
            out=dbg["idx"][:], in_=idx_i[:].rearrange("p b l -> p (b l)")
        )
        nc.sync.dma_start(
            out=dbg["patch0"][:], in_=patch[0][:].rearrange("p b r -> p (b r)")
        )
        nc.sync.dma_start(
            out=dbg["patch3"][:], in_=patch[3][:].rearrange("p b r -> p (b r)")
        )
        nc.sync.dma_start(
            out=dbg["wx0"][:], in_=wx0e[:].rearrange("p b l k -> p (b l k)")
        )
        nc.sync.dma_start(
            out=dbg["cv0"][:],
            in_=cv_dram[0][0 : PAD + 2 * LVLSUM].rearrange("(p x) -> p x", p=128),
        )
        nc.sync.dma_start(
            out=dbg["outq"][:], in_=outq[:].rearrange("p b l dy dx -> p (b l dy dx)")
        )


def build_program(debug=False):
    """Build (once) the single-core SPMD bass program."""
    key = ("nc", debug)
    if key in _CACHE:
        return _CACHE[key]
    import concourse.tile as tile
    import concourse.mybir as mybir
    from concourse import bacc

    f32 = mybir.dt.float32
    bf16 = mybir.dt.bfloat16
    i32 = mybir.dt.int32
    nc = bacc.Bacc(
        "TRN2",
        target_bir_lowering=False,
        debug=False,
        enable_asserts=True,
        num_devices=NCORES,
    )
    f1c = nc.dram_tensor("f1c", [D, QPC], bf16, kind="ExternalInput").ap()
    f2 = nc.dram_tensor("f2", [D, H * W], bf16, kind="ExternalInput").ap()
    crd = nc.dram_tensor("crd", [2, QPC], f32, kind="ExternalInput").ap()
    out = nc.dram_tensor("out", [NCH, H // NCORES, W], f32, kind="ExternalOutput").ap()
    dbg = None
    if debug:
        dbg = {
            "idx": nc.dram_tensor(
                "dbg_idx", [128, NBLK * NUM_LEVELS], i32, kind="ExternalOutput"
            ).ap(),
            "patch0": nc.dram_tensor(
                "dbg_patch0", [128, NBLK * ROWL[0]], bf16, kind="ExternalOutput"
            ).ap(),
            "patch3": nc.dram_tensor(
                "dbg_patch3", [128, NBLK * ROWL[3]], bf16, kind="ExternalOutput"
            ).ap(),
            "wx0": nc.dram_tensor(
                "dbg_wx0", [128, NBLK * NUM_LEVELS * KK], bf16, kind="ExternalOutput"
            ).ap(),
            "cv0": nc.dram_tensor(
                "dbg_cv0", [128, (PAD + 2 * LVLSUM) // 128], bf16,
                kind="ExternalOutput",
            ).ap(),
            "outq": nc.dram_tensor(
                "dbg_outq", [128, NBLK * NCH], bf16, kind="ExternalOutput"
            ).ap(),
        }

    from contextlib import ExitStack

    with tile.TileContext(nc) as tc, ExitStack() as ctx:
        _emit(ctx, tc, out, f1c, f2, crd, dbg=dbg)
    nc.compile()
    _CACHE[key] = nc
    return nc


def make_in_maps(fmap1, fmap2, coords):
    import ml_dtypes

    bf = ml_dtypes.bfloat16
    f1 = np.ascontiguousarray(
        np.asarray(fmap1, dtype=np.float32).reshape(D, H * W)
    ).astype(bf)
    f2 = np.ascontiguousarray(
        np.asarray(fmap2, dtype=np.float32).reshape(D, H * W)
    ).astype(bf)
    crd = np.asarray(coords, dtype=np.float32).reshape(2, H * W)
    in_maps = []
    for c in range(NCORES):
        sl = slice(c * QPC, (c + 1) * QPC)
        in_maps.append(
            {
                "f1c": np.ascontiguousarray(f1[:, sl]),
                "f2": f2,
                "crd": np.ascontiguousarray(crd[:, sl]),
            }
        )
    return in_maps


def kernel(fmap1, fmap2, coords):
    from concourse.bass_utils import run_bass_kernel_spmd

    nc = build_program()
    in_maps = make_in_maps(fmap1, fmap2, coords)
    res = run_bass_kernel_spmd(nc, in_maps, list(range(NCORES)))
    parts = [res.results[c]["out"] for c in range(NCORES)]  # [324, 8, 128] each
    full = np.concatenate(parts, axis=1)  # [324, 64, 128]
    return full[None].astype(np.float32)


# revision 10
# speedup vs baseline: 1.1136x; 1.1136x over previous
# Correlation2D (RAFT-style correlation pyramid lookup) on 8 TRN2 NeuronCores.
#
# Sharding: data-parallel over the bs*h*w query axis. Each core owns 1024
# queries (= 8 image rows). Per block of 128 queries it computes its slice of
# the cost volume via a bf16 GEMM (fmap2 replicated, pooling folded into
# fmap2), writes the 4-level pyramid per-query-contiguous to DRAM (bf16), and
# gathers ONE contiguous run per (block, level) spanning the whole 10x10
# patch (rows are Wl apart inside the run; the bilinear stage reads the run
# through a strided view). Bilinear combine is separable in bf16, spread
# across the GEMM pipeline; output is PE-transposed to channel-major.
# Output per core is [324, 8, 128] f32, host concatenates along y.
#
# Scheduling notes (engine streams are in-order, so emission order matters):
# - f2 loads in 4 column sub-chunks; block0/block1 level-0 GEMMs run while
#   the rest of f2 streams in; f2 pooling in two row-chains gated on subloads.
# - matmuls grouped 4 N-chunks per K-half so walrus can reuse LDWEIGHTS.
# - PSUM drains alternate vector/scalar; weights/idx calc sits between
#   early drains; bilinear is emitted 2 levels at a time after gathers.
import numpy as np

# ---- problem constants (hardcoded per contest contract) ----
H, W = 64, 128
D = 256
NUM_LEVELS = 4
RADIUS = 4
KK = 2 * RADIUS + 1        # 9
PS = KK + 1                # 10x10 patch per (query, level)
NCORES = 8
QPC = (H * W) // NCORES    # 1024 queries per core
NBLK = QPC // 128          # 8 blocks of 128 queries
LVL_W = [W >> l for l in range(NUM_LEVELS)]            # 128 64 32 16
LVL_H = [H >> l for l in range(NUM_LEVELS)]            # 64 32 16 8
LVL_N = [LVL_W[l] * LVL_H[l] for l in range(NUM_LEVELS)]   # 8192 2048 512 128
LVL_OFF = [sum(LVL_N[:l]) for l in range(NUM_LEVELS)]  # 0 8192 10240 10752
LVLSUM = sum(LVL_N)        # 10880
RUN = [KK * LVL_W[l] + PS for l in range(NUM_LEVELS)]  # 1162 586 298 154
ROWL = [PS * LVL_W[l] for l in range(NUM_LEVELS)]      # 1280 640 320 160
PAD = 1024                 # zeroed head/tail pad (elements) per block buffer
BQS = 128 * LVLSUM         # elements of cv per block
BTOT = PAD + BQS + PAD     # per-block DRAM tensor elements (bf16)
NCH = NUM_LEVELS * KK * KK  # 324 output channels
MM_N = 512                 # matmul N-chunk (one PSUM bank of f32)
GRP = 4                    # N-chunks per weight-load group
CHUNKS = [(0, 128), (128, 128), (256, NCH - 256)]  # output channel chunks

_CACHE = {}


def _emit(ctx, tc, out_ext, f1c, f2, crd, dbg=None):
    import concourse.bass as bass
    import concourse.mybir as mybir
    from concourse.masks import make_identity

    nc = tc.nc
    f32 = mybir.dt.float32
    bf16 = mybir.dt.bfloat16
    i32 = mybir.dt.int32
    Alu = mybir.AluOpType

    const_pool = ctx.enter_context(tc.tile_pool(name="constp", bufs=1))
    f2_pool = ctx.enter_context(tc.tile_pool(name="f2p", bufs=1))
    f1_pool = ctx.enter_context(tc.tile_pool(name="f1p", bufs=1))
    coordp = ctx.enter_context(tc.tile_pool(name="coordp", bufs=1))
    small = ctx.enter_context(tc.tile_pool(name="small", bufs=2))
    cvp = ctx.enter_context(tc.tile_pool(name="cvp", bufs=2))
    patchp = ctx.enter_context(tc.tile_pool(name="patchp", bufs=1))
    txp = ctx.enter_context(tc.tile_pool(name="txp", bufs=2))
    outp = ctx.enter_context(tc.tile_pool(name="outp", bufs=1))
    psum = ctx.enter_context(tc.tile_pool(name="psum", bufs=6, space="PSUM"))
    psum_t = ctx.enter_context(tc.tile_pool(name="psumt", bufs=2, space="PSUM"))
    dramp = ctx.enter_context(tc.tile_pool(name="dramp", bufs=1, space="DRAM"))

    # ------------- per-block DRAM cv buffers (query-contiguous pyramid) ----
    cv_dram = [dramp.tile([BTOT], bf16, name=f"cv_dram{b}") for b in range(NBLK)]

    # ---------------- input loads (ordered: small/critical first) ----------
    cxs = coordp.tile([128, NBLK], f32, name="cxs")
    cys = coordp.tile([128, NBLK], f32, name="cys")
    nc.scalar.dma_start(out=cxs[:], in_=crd[0, :].rearrange("(b p) -> p b", p=128))
    nc.scalar.dma_start(out=cys[:], in_=crd[1, :].rearrange("(b p) -> p b", p=128))

    f1h = []
    for k in range(2):
        t = f1_pool.tile([128, QPC], bf16, name=f"f1h{k}")
        nc.sync.dma_start(out=t[:], in_=f1c[k * 128 : (k + 1) * 128, :])
        f1h.append(t)

    # f2 halves in 4 column sub-loads each so early GEMM chunks start early
    NSUB = 4
    SUBW = LVL_N[0] // NSUB  # 2048
    f2_halves = []
    for k in range(2):
        f2h = f2_pool.tile([128, LVL_N[0]], bf16, name=f"f2h{k}")
        f2_halves.append(f2h)
    for s in range(NSUB):
        for k in range(2):
            nc.sync.dma_start(
                out=f2_halves[k][:, s * SUBW : (s + 1) * SUBW],
                in_=f2[k * 128 : (k + 1) * 128, s * SUBW : (s + 1) * SUBW],
            )

    # zero head/tail pads (gather runs can poke into them; must stay finite)
    ztile = const_pool.tile([128, 8], bf16, name="ztile")
    nc.gpsimd.memset(ztile[:], 0.0)
    for b in range(NBLK):
        nc.sync.dma_start(
            out=cv_dram[b][0:PAD].rearrange("(p x) -> p x", p=128), in_=ztile[:]
        )
        nc.sync.dma_start(
            out=cv_dram[b][PAD + BQS : BTOT].rearrange("(p x) -> p x", p=128),
            in_=ztile[:],
        )

    # ---------------- constants -------------------------------------------
    # per-level vectors [128, 4] (exact values via memset per lane)
    invv = const_pool.tile([128, NUM_LEVELS], f32, name="invv")
    wlv = const_pool.tile([128, NUM_LEVELS], f32, name="wlv")
    limxv = const_pool.tile([128, NUM_LEVELS], f32, name="limxv")
    limyv = const_pool.tile([128, NUM_LEVELS], f32, name="limyv")
    cvecv = const_pool.tile([128, NUM_LEVELS], f32, name="cvecv")
    for l in range(NUM_LEVELS):
        nc.gpsimd.memset(invv[:, l : l + 1], 1.0 / (1 << l))
        nc.gpsimd.memset(wlv[:, l : l + 1], float(LVL_W[l]))
        nc.gpsimd.memset(limxv[:, l : l + 1], float(LVL_W[l] - 1))
        nc.gpsimd.memset(limyv[:, l : l + 1], float(LVL_H[l] - 1))
        nc.gpsimd.memset(
            cvecv[:, l : l + 1],
            float(PAD + LVL_OFF[l] - RADIUS * LVL_W[l] - RADIUS),
        )

    # c ramp: -4..5 (patch-col -> absolute offset from floor(coord))
    cramp_i = const_pool.tile([128, PS], i32, name="cramp_i")
    nc.gpsimd.iota(cramp_i[:], pattern=[[1, PS]], base=-RADIUS, channel_multiplier=0)
    crampf = const_pool.tile([128, PS], f32, name="crampf")
    nc.gpsimd.tensor_copy(out=crampf[:], in_=cramp_i[:])

    # per-query element base offset of its pyramid inside its block's buffer
    pq_i = coordp.tile([128, 1], i32, name="pq_i")
    nc.gpsimd.iota(pq_i[:], pattern=[[1, 1]], base=0, channel_multiplier=1)
    bqf = coordp.tile([128, 1], f32, name="bqf")
    nc.gpsimd.tensor_copy(out=bqf[:], in_=pq_i[:])
    nc.gpsimd.tensor_scalar_mul(bqf[:], bqf[:], float(LVLSUM))

    ident = const_pool.tile([128, 128], bf16, name="ident")
    make_identity(nc, ident[:])

    # weight / index tiles
    idx_i = coordp.tile([128, NBLK, NUM_LEVELS], i32, name="idx_i")
    wx0e = coordp.tile([128, NBLK, NUM_LEVELS, KK], bf16, name="wx0e")
    wx1e = coordp.tile([128, NBLK, NUM_LEVELS, KK], bf16, name="wx1e")
    wy0e = coordp.tile([128, NBLK, NUM_LEVELS, KK], bf16, name="wy0e")
    wy1e = coordp.tile([128, NBLK, NUM_LEVELS, KK], bf16, name="wy1e")

    sh3 = [128, NBLK, NUM_LEVELS]
    ixf = coordp.tile(sh3, f32, name="ixf")
    iyf = coordp.tile(sh3, f32, name="iyf")
    fxe = coordp.tile(sh3, f32, name="fxe")
    fye = coordp.tile(sh3, f32, name="fye")

    def emit_idx():
        """floor/frac for all levels + gather start indices (vector)."""
        for (src, ff, fr, nm) in ((cxs, ixf, fxe, "x"), (cys, iyf, fye, "y")):
            # xs = src / 2^l; floor via i32 cast (rounds on HW) + is_gt fix
            xs = small.tile(sh3, f32, name=f"xs_{nm}", tag="xs")
            nc.vector.tensor_tensor(
                xs[:],
                src[:].unsqueeze(2).to_broadcast(sh3),
                invv[:].unsqueeze(1).to_broadcast(sh3),
                op=Alu.mult,
            )
            ii = small.tile(sh3, i32, name=f"ii_{nm}", tag="ii")
            nc.vector.tensor_copy(out=ii[:], in_=xs[:])
            nc.vector.tensor_copy(out=ff[:], in_=ii[:])
            adj = small.tile(sh3, f32, name=f"adj_{nm}", tag="adj")
            nc.vector.tensor_tensor(adj[:], ff[:], xs[:], op=Alu.is_gt)
            nc.vector.tensor_tensor(ff[:], ff[:], adj[:], op=Alu.subtract)
            nc.vector.tensor_tensor(fr[:], xs[:], ff[:], op=Alu.subtract)

        # gather run start: PAD + p*LVLSUM + lvl_off + (iy-4)*Wl + (ix-4)
        t1 = small.tile(sh3, f32, name="t1", tag="t1")
        nc.vector.tensor_tensor(
            t1[:], iyf[:], wlv[:].unsqueeze(1).to_broadcast(sh3), op=Alu.mult
        )
        nc.vector.tensor_tensor(t1[:], t1[:], ixf[:], op=Alu.add)
        nc.vector.tensor_tensor(
            t1[:], t1[:], bqf[:].unsqueeze(2).to_broadcast(sh3), op=Alu.add
        )
        nc.vector.tensor_tensor(
            t1[:], t1[:], cvecv[:].unsqueeze(1).to_broadcast(sh3), op=Alu.add
        )
        nc.vector.tensor_copy(out=idx_i[:], in_=t1[:])  # exact ints

    def emit_weights():
        """Bilinear weights with OOB masks, all levels batched (vector)."""
        sh4 = [128, NBLK, NUM_LEVELS, PS]
        shk = [128, NBLK, NUM_LEVELS, KK]
        for (w0t, w1t, frac, posf, limv) in (
            (wx0e, wx1e, fxe, ixf, limxv),
            (wy0e, wy1e, fye, iyf, limyv),
        ):
            pos = small.tile(sh4, f32, name="pos", tag="pos")
            nc.vector.tensor_tensor(
                pos[:],
                posf[:].unsqueeze(3).to_broadcast(sh4),
                crampf[:].unsqueeze(1).unsqueeze(1).to_broadcast(sh4),
                op=Alu.add,
            )
            # in-bounds <=> |2*pos - lim| <= lim
            nc.vector.tensor_scalar_mul(pos[:], pos[:], 2.0)
            nc.vector.tensor_tensor(
                pos[:], pos[:],
                limv[:].unsqueeze(1).unsqueeze(3).to_broadcast(sh4),
                op=Alu.subtract,
            )
            ok = small.tile(sh4, f32, name="ok", tag="ok")
            nc.scalar.activation(ok[:], pos[:], mybir.ActivationFunctionType.Abs)
            nc.vector.tensor_tensor(
                ok[:], ok[:],
                limv[:].unsqueeze(1).unsqueeze(3).to_broadcast(sh4),
                op=Alu.is_le,
            )
            w0 = small.tile(sh3, f32, name="w0", tag="w0")
            nc.vector.tensor_scalar(w0[:], frac[:], -1.0, 1.0,
                                    op0=Alu.mult, op1=Alu.add)  # 1 - frac
            nc.vector.tensor_tensor(
                w0t[:], w0[:].unsqueeze(3).to_broadcast(shk),
                ok[:, :, :, 0:KK], op=Alu.mult,
            )
            nc.vector.tensor_tensor(
                w1t[:], frac[:].unsqueeze(3).to_broadcast(shk),
                ok[:, :, :, 1:PS], op=Alu.mult,
            )

    # ---------------- fmap2 pyramid pooling (two row-chains) ---------------
    # pooled levels keep raw SUMS; the 1/16 * 0.25^l scale is in the drain.
    # chain c covers output rows [c*Hl/2, (c+1)*Hl/2) of every level; chain 0
    # only needs f2 columns 0:4096 (sub-loads 0-1), chain 1 the rest.
    f2_lv = [f2_halves]
    for l in range(1, NUM_LEVELS):
        Wl, Hl = LVL_W[l], LVL_H[l]
        f2_lv.append(
            [f2_pool.tile([128, Hl * Wl], bf16, name=f"f2l{l}_{k}") for k in range(2)]
        )

    def emit_pool_chain(c):
        for l in range(1, NUM_LEVELS):
            Wl, Hl = LVL_W[l], LVL_H[l]
            pw, ph = LVL_W[l - 1], LVL_H[l - 1]
            r0, r1 = c * ph // 2, (c + 1) * ph // 2  # prev-level row range
            for k in range(2):
                prev = f2_lv[l - 1][k][:].rearrange(
                    "p (h w two) -> p h w two", h=ph, w=pw // 2, two=2
                )
                s1 = small.tile(
                    [128, ph // 2, pw // 2], bf16, name=f"s1_{c}_{l}_{k}",
                    tag="poolt", bufs=1,
                )
                nc.vector.tensor_tensor(
                    s1[:], prev[:, r0:r1, :, 0], prev[:, r0:r1, :, 1], op=Alu.add
                )
                s1v = s1[:].rearrange(
                    "p (h2 two) w -> p h2 two w", h2=Hl // 2, two=2
                )
                curv = f2_lv[l][k][:].rearrange("p (h w) -> p h w", h=Hl, w=Wl)
                nc.vector.tensor_tensor(
                    curv[:, c * Hl // 2 : (c + 1) * Hl // 2, :],
                    s1v[:, :, 0, :], s1v[:, :, 1, :], op=Alu.add,
                )

    # ---------------- patch tiles (gather destinations) --------------------
    patch = [
        patchp.tile([128, NBLK, ROWL[l]], bf16, name=f"patch{l}")
        for l in range(NUM_LEVELS)
    ]
    outq = patchp.tile([128, NBLK, NUM_LEVELS, KK, KK], bf16, name="outq")
    outq_v = outq[:].rearrange("p b l dy dx -> p b (l dy dx)")
    outT = [
        outp.tile([128, NBLK, 128], f32, name=f"outT{k}")
        for k in range(len(CHUNKS))
    ]

    # ---------------- GEMM helpers -----------------------------------------
    drain_parity = [0]

    def emit_gemm(b, lvls, cv_sb):
        chunks = [
            (l, n0, min(LVL_N[l], n0 + MM_N))
            for l in lvls
            for n0 in range(0, LVL_N[l], MM_N)
        ]
        for g0 in range(0, len(chunks), GRP):
            grp = chunks[g0 : g0 + GRP]
            pts = [
                psum.tile([128, n1 - n0], f32, name="pt", tag="pt")
                for (_, n0, n1) in grp
            ]
            # one weight-load per K-half per group (same lhsT across chunks)
            for k in range(2):
                for pt, (l, n0, n1) in zip(pts, grp):
                    nc.tensor.matmul(
                        pt[:],
                        f1h[k][:, b * 128 : (b + 1) * 128],
                        f2_lv[l][k][:, n0:n1],
                        start=(k == 0),
                        stop=(k == 1),
                    )
            for pt, (l, n0, n1) in zip(pts, grp):
                scale_l = (1.0 / 16.0) * (0.25 ** l)
                dst = cv_sb[:, LVL_OFF[l] + n0 : LVL_OFF[l] + n1]
                if drain_parity[0] % 2 == 0:
                    nc.vector.tensor_scalar_mul(dst[:], pt[:], scale_l)
                else:
                    nc.scalar.mul(dst[:], pt[:], scale_l)
                drain_parity[0] += 1

    def emit_write_gather(b, cv_sb):
        # one DMA for the whole block's cv (rows 21.76KB contiguous)
        nc.sync.dma_start(
            out=cv_dram[b][PAD : PAD + BQS].rearrange("(q s) -> q s", s=LVLSUM),
            in_=cv_sb[:],
        )
        # one indirect gather per level: contiguous run covering the patch
        cv2d = cv_dram[b][:].rearrange("(a x) -> a x", a=1024)
        for l in range(NUM_LEVELS):
            nc.gpsimd.indirect_dma_start(
                out=patch[l][:, b, 0 : RUN[l]],
                out_offset=None,
                in_=cv2d,
                in_offset=bass.IndirectOffsetOnAxis(
                    ap=idx_i[:, b, l].unsqueeze(1), axis=1
                ),
            )

    def emit_bilinear(h, lvls):
        """Bilinear for blocks [4h, 4h+4), given levels (vector)."""
        b0, b1 = 4 * h, 4 * h + 4
        nb = b1 - b0
        for l in lvls:
            Wl = LVL_W[l]
            Pv = patch[l][:].rearrange("p b (r c) -> p b r c", r=PS, c=Wl)
            bshape_x = [128, nb, PS, KK]
            tx = txp.tile([128, nb, PS, KK], bf16, name=f"tx{h}{l}", tag="tx")
            tx2 = txp.tile([128, nb, PS, KK], bf16, name=f"tx2{h}{l}", tag="tx2")
            nc.vector.tensor_tensor(
                tx[:], Pv[:, b0:b1, :, 0:KK],
                wx0e[:, b0:b1, l, :].unsqueeze(2).to_broadcast(bshape_x),
                op=Alu.mult,
            )
            nc.vector.tensor_tensor(
                tx2[:], Pv[:, b0:b1, :, 1:PS],
                wx1e[:, b0:b1, l, :].unsqueeze(2).to_broadcast(bshape_x),
                op=Alu.mult,
            )
            nc.vector.tensor_tensor(tx[:], tx[:], tx2[:], op=Alu.add)

            bshape_y = [128, nb, KK, KK]
            oq2 = txp.tile([128, nb, KK, KK], bf16, name=f"oq2{h}{l}", tag="oq2")
            nc.vector.tensor_tensor(
                oq2[:], tx[:, :, 0:KK, :],
                wy0e[:, b0:b1, l, :].unsqueeze(3).to_broadcast(bshape_y),
                op=Alu.mult,
            )
            nc.vector.tensor_tensor(
                outq[:, b0:b1, l], tx[:, :, 1:PS, :],
                wy1e[:, b0:b1, l, :].unsqueeze(3).to_broadcast(bshape_y),
                op=Alu.mult,
            )
            nc.vector.tensor_tensor(
                outq[:, b0:b1, l], outq[:, b0:b1, l], oq2[:], op=Alu.add
            )

    def emit_transpose_out(h):
        b0, b1 = 4 * h, 4 * h + 4
        for k, (c0, nk) in enumerate(CHUNKS):
            for b in range(b0, b1):
                ptt = psum_t.tile([128, 128], bf16, name="ptt", tag="ptt")
                nc.tensor.transpose(
                    out=ptt[:nk, :], in_=outq_v[:, b, c0 : c0 + nk],
                    identity=ident[:],
                )
                if b % 2 == 0:
                    nc.vector.tensor_copy(out=outT[k][0:nk, b, :], in_=ptt[:nk, :])
                else:
                    nc.scalar.copy(out=outT[k][0:nk, b, :], in_=ptt[:nk, :])
        for k, (c0, nk) in enumerate(CHUNKS):
            nc.sync.dma_start(
                out=out_ext[c0 : c0 + nk, b0:b1, :], in_=outT[k][0:nk, b0:b1, :]
            )

    # ---------------- schedule ---------------------------------------------
    emit_pool_chain(0)                     # gated on f2 sub-loads 0-1
    cv_sb0 = cvp.tile([128, LVLSUM], bf16, name="cv_sb", tag="cv_sb")
    emit_gemm(0, [0], cv_sb0)              # L0 GEMM while f2 tail streams in
    cv_sb1 = cvp.tile([128, LVLSUM], bf16, name="cv_sb", tag="cv_sb")
    emit_gemm(1, [0], cv_sb1)
    emit_pool_chain(1)                     # gated on f2 sub-loads 2-3
    emit_idx()
    emit_gemm(0, [1, 2, 3], cv_sb0)
    emit_write_gather(0, cv_sb0)
    emit_weights()
    emit_gemm(1, [1, 2, 3], cv_sb1)
    emit_write_gather(1, cv_sb1)
    for b in range(2, NBLK):
        cv_sb = cvp.tile([128, LVLSUM], bf16, name="cv_sb", tag="cv_sb")
        emit_gemm(b, [0, 1, 2, 3], cv_sb)
        emit_write_gather(b, cv_sb)
        if b == 3:
            emit_bilinear(0, [0, 1])
        elif b == 4:
            emit_bilinear(0, [2, 3])
        elif b == 5:
            emit_transpose_out(0)
    emit_bilinear(1, [0, 1, 2, 3])
    emit_transpose_out(1)

    if dbg is not None:
        nc.sync.dma_start(
            out=dbg["idx"][:], in_=idx_i[:].rearrange("p b l -> p (b l)")
        )
        nc.sync.dma_start(
            out=dbg["patch0"][:], in_=patch[0][:].rearrange("p b r -> p (b r)")
        )
        nc.sync.dma_start(
            out=dbg["patch3"][:], in_=patch[3][:].rearrange("p b r -> p (b r)")
        )
        nc.sync.dma_start(
            out=dbg["wx0"][:], in_=wx0e[:].rearrange("p b l k -> p (b l k)")
        )
        nc.sync.dma_start(
            out=dbg["cv0"][:],
            in_=cv_dram[0][0 : PAD + 2 * LVLSUM].rearrange("(p x) -> p x", p=128),
        )
        nc.sync.dma_start(
            out=dbg["outq"][:], in_=outq[:].rearrange("p b l dy dx -> p (b l dy dx)")
        )


def build_program(debug=False):
    """Build (once) the single-core SPMD bass program."""
    key = ("nc", debug)
    if key in _CACHE:
        return _CACHE[key]
    import concourse.tile as tile
    import concourse.mybir as mybir
    from concourse import bacc

    f32 = mybir.dt.float32
    bf16 = mybir.dt.bfloat16
    i32 = mybir.dt.int32
    nc = bacc.Bacc(
        "TRN2",
        target_bir_lowering=False,
        debug=False,
        enable_asserts=True,
        num_devices=NCORES,
    )
    f1c = nc.dram_tensor("f1c", [D, QPC], bf16, kind="ExternalInput").ap()
    f2 = nc.dram_tensor("f2", [D, H * W], bf16, kind="ExternalInput").ap()
    crd = nc.dram_tensor("crd", [2, QPC], f32, kind="ExternalInput").ap()
    out = nc.dram_tensor("out", [NCH, H // NCORES, W], f32, kind="ExternalOutput").ap()
    dbg = None
    if debug:
        dbg = {
            "idx": nc.dram_tensor(
                "dbg_idx", [128, NBLK * NUM_LEVELS], i32, kind="ExternalOutput"
            ).ap(),
            "patch0": nc.dram_tensor(
                "dbg_patch0", [128, NBLK * ROWL[0]], bf16, kind="ExternalOutput"
            ).ap(),
            "patch3": nc.dram_tensor(
                "dbg_patch3", [128, NBLK * ROWL[3]], bf16, kind="ExternalOutput"
            ).ap(),
            "wx0": nc.dram_tensor(
                "dbg_wx0", [128, NBLK * NUM_LEVELS * KK], bf16, kind="ExternalOutput"
            ).ap(),
            "cv0": nc.dram_tensor(
                "dbg_cv0", [128, (PAD + 2 * LVLSUM) // 128], bf16,
                kind="ExternalOutput",
            ).ap(),
            "outq": nc.dram_tensor(
                "dbg_outq", [128, NBLK * NCH], bf16, kind="ExternalOutput"
            ).ap(),
        }

    from contextlib import ExitStack

    with tile.TileContext(nc) as tc, ExitStack() as ctx:
        _emit(ctx, tc, out, f1c, f2, crd, dbg=dbg)
    nc.compile()
    _CACHE[key] = nc
    return nc


def make_in_maps(fmap1, fmap2, coords):
    import ml_dtypes

    bf = ml_dtypes.bfloat16
    f1 = np.ascontiguousarray(
        np.asarray(fmap1, dtype=np.float32).reshape(D, H * W)
    ).astype(bf)
    f2 = np.ascontiguousarray(
        np.asarray(fmap2, dtype=np.float32).reshape(D, H * W)
    ).astype(bf)
    crd = np.asarray(coords, dtype=np.float32).reshape(2, H * W)
    in_maps = []
    for c in range(NCORES):
        sl = slice(c * QPC, (c + 1) * QPC)
        in_maps.append(
            {
                "f1c": np.ascontiguousarray(f1[:, sl]),
                "f2": f2,
                "crd": np.ascontiguousarray(crd[:, sl]),
            }
        )
    return in_maps


def kernel(fmap1, fmap2, coords):
    from concourse.bass_utils import run_bass_kernel_spmd

    nc = build_program()
    in_maps = make_in_maps(fmap1, fmap2, coords)
    res = run_bass_kernel_spmd(nc, in_maps, list(range(NCORES)))
    parts = [res.results[c]["out"] for c in range(NCORES)]  # [324, 8, 128] each
    full = np.concatenate(parts, axis=1)  # [324, 64, 128]
    return full[None].astype(np.float32)


# revision 16
# speedup vs baseline: 1.1501x; 1.0328x over previous
# Correlation2D (RAFT-style correlation pyramid lookup) on 8 TRN2 NeuronCores.
#
# Sharding: data-parallel over the bs*h*w query axis. Each core owns 1024
# queries (= 8 image rows). Per block of 128 queries it computes its slice of
# the cost volume via a bf16 GEMM (fmap2 replicated, pooling folded into
# fmap2), writes the 4-level pyramid per-query-contiguous to DRAM (bf16), and
# gathers ONE contiguous run per (block, level) spanning the whole 10x10
# patch (rows are Wl apart inside the run; the bilinear stage reads the run
# through a strided view). Bilinear combine is separable in bf16, spread
# across the GEMM pipeline; output is PE-transposed to channel-major.
# Output per core is [324, 8, 128] f32, host concatenates along y.
#
# Scheduling notes (engine streams are in-order, so emission order matters):
# - f2 loads in 4 column sub-chunks; block0/block1 level-0 GEMMs run while
#   the rest of f2 streams in; f2 pooling in two row-chains gated on subloads.
# - matmuls grouped 4 N-chunks per K-half so walrus can reuse LDWEIGHTS.
# - PSUM drains alternate vector/scalar; weights/idx calc sits between
#   early drains; bilinear is emitted 2 levels at a time after gathers.
import numpy as np

# ---- problem constants (hardcoded per contest contract) ----
H, W = 64, 128
D = 256
NUM_LEVELS = 4
RADIUS = 4
KK = 2 * RADIUS + 1        # 9
PS = KK + 1                # 10x10 patch per (query, level)
NCORES = 8
QPC = (H * W) // NCORES    # 1024 queries per core
NBLK = QPC // 128          # 8 blocks of 128 queries
LVL_W = [W >> l for l in range(NUM_LEVELS)]            # 128 64 32 16
LVL_H = [H >> l for l in range(NUM_LEVELS)]            # 64 32 16 8
LVL_N = [LVL_W[l] * LVL_H[l] for l in range(NUM_LEVELS)]   # 8192 2048 512 128
LVL_OFF = [sum(LVL_N[:l]) for l in range(NUM_LEVELS)]  # 0 8192 10240 10752
LVLSUM = sum(LVL_N)        # 10880
RUN = [KK * LVL_W[l] + PS for l in range(NUM_LEVELS)]  # 1162 586 298 154
ROWL = [PS * LVL_W[l] for l in range(NUM_LEVELS)]      # 1280 640 320 160
PAD = 1024                 # zeroed head/tail pad (elements) per block buffer
BQS = 128 * LVLSUM         # elements of cv per block
BTOT = PAD + BQS + PAD     # per-block DRAM tensor elements (bf16)
NCH = NUM_LEVELS * KK * KK  # 324 output channels
MM_N = 512                 # matmul N-chunk (one PSUM bank of f32)
GRP = 4                    # N-chunks per weight-load group
CHUNKS = [(0, 128), (128, 128), (256, NCH - 256)]  # output channel chunks

_CACHE = {}


def _emit(ctx, tc, out_ext, f1c, f2, crd, dbg=None):
    import concourse.bass as bass
    import concourse.mybir as mybir
    from concourse.masks import make_identity

    nc = tc.nc
    f32 = mybir.dt.float32
    bf16 = mybir.dt.bfloat16
    i32 = mybir.dt.int32
    Alu = mybir.AluOpType

    const_pool = ctx.enter_context(tc.tile_pool(name="constp", bufs=1))
    f2_pool = ctx.enter_context(tc.tile_pool(name="f2p", bufs=1))
    f1_pool = ctx.enter_context(tc.tile_pool(name="f1p", bufs=1))
    coordp = ctx.enter_context(tc.tile_pool(name="coordp", bufs=1))
    small = ctx.enter_context(tc.tile_pool(name="small", bufs=2))
    cvp = ctx.enter_context(tc.tile_pool(name="cvp", bufs=3))
    patchp = ctx.enter_context(tc.tile_pool(name="patchp", bufs=1))
    txp = ctx.enter_context(tc.tile_pool(name="txp", bufs=2))
    outp = ctx.enter_context(tc.tile_pool(name="outp", bufs=1))
    # pt1024 spans 2 PSUM banks (matmuls stay within one bank each); the
    # L2+L3 tail reuses the same slots (512 in bank a, 128 in bank b)
    psum = ctx.enter_context(tc.tile_pool(name="psum", bufs=3, space="PSUM"))
    psum_t = ctx.enter_context(tc.tile_pool(name="psumt", bufs=2, space="PSUM"))
    dramp = ctx.enter_context(tc.tile_pool(name="dramp", bufs=1, space="DRAM"))

    # ------------- per-block DRAM cv buffers (query-contiguous pyramid) ----
    cv_dram = [dramp.tile([BTOT], bf16, name=f"cv_dram{b}") for b in range(NBLK)]

    # ---------------- input loads (ordered: small/critical first) ----------
    cxs = coordp.tile([128, NBLK], f32, name="cxs")
    cys = coordp.tile([128, NBLK], f32, name="cys")
    nc.scalar.dma_start(out=cxs[:], in_=crd[0, :].rearrange("(b p) -> p b", p=128))
    nc.scalar.dma_start(out=cys[:], in_=crd[1, :].rearrange("(b p) -> p b", p=128))

    f1h = []
    for k in range(2):
        t = f1_pool.tile([128, QPC], bf16, name=f"f1h{k}")
        nc.sync.dma_start(out=t[:], in_=f1c[k * 128 : (k + 1) * 128, :])
        f1h.append(t)

    # f2 halves in 4 column sub-loads each so early GEMM chunks start early
    NSUB = 4
    SUBW = LVL_N[0] // NSUB  # 2048
    f2_halves = []
    for k in range(2):
        f2h = f2_pool.tile([128, LVL_N[0]], bf16, name=f"f2h{k}")
        f2_halves.append(f2h)
    for s in range(NSUB):
        for k in range(2):
            nc.sync.dma_start(
                out=f2_halves[k][:, s * SUBW : (s + 1) * SUBW],
                in_=f2[k * 128 : (k + 1) * 128, s * SUBW : (s + 1) * SUBW],
            )

    # zero head/tail pads (gather runs can poke into them; must stay finite)
    ztile = const_pool.tile([128, 8], bf16, name="ztile")
    nc.gpsimd.memset(ztile[:], 0.0)
    for b in range(NBLK):
        nc.sync.dma_start(
            out=cv_dram[b][0:PAD].rearrange("(p x) -> p x", p=128), in_=ztile[:]
        )
        nc.sync.dma_start(
            out=cv_dram[b][PAD + BQS : BTOT].rearrange("(p x) -> p x", p=128),
            in_=ztile[:],
        )

    # ---------------- constants -------------------------------------------
    # per-level vectors [128, 4] (exact values via memset per lane)
    invv = const_pool.tile([128, NUM_LEVELS], f32, name="invv")
    wlv = const_pool.tile([128, NUM_LEVELS], f32, name="wlv")
    limxv = const_pool.tile([128, NUM_LEVELS], f32, name="limxv")
    limyv = const_pool.tile([128, NUM_LEVELS], f32, name="limyv")
    cvecv = const_pool.tile([128, NUM_LEVELS], f32, name="cvecv")
    for l in range(NUM_LEVELS):
        nc.gpsimd.memset(invv[:, l : l + 1], 1.0 / (1 << l))
        nc.gpsimd.memset(wlv[:, l : l + 1], float(LVL_W[l]))
        nc.gpsimd.memset(limxv[:, l : l + 1], float(LVL_W[l] - 1))
        nc.gpsimd.memset(limyv[:, l : l + 1], float(LVL_H[l] - 1))
        nc.gpsimd.memset(
            cvecv[:, l : l + 1],
            float(PAD + LVL_OFF[l] - RADIUS * LVL_W[l] - RADIUS),
        )

    # c ramp: -4..5 (patch-col -> absolute offset from floor(coord))
    cramp_i = const_pool.tile([128, PS], i32, name="cramp_i")
    nc.gpsimd.iota(cramp_i[:], pattern=[[1, PS]], base=-RADIUS, channel_multiplier=0)
    crampf = const_pool.tile([128, PS], f32, name="crampf")
    nc.gpsimd.tensor_copy(out=crampf[:], in_=cramp_i[:])

    # per-query element base offset of its pyramid inside its block's buffer
    pq_i = coordp.tile([128, 1], i32, name="pq_i")
    nc.gpsimd.iota(pq_i[:], pattern=[[1, 1]], base=0, channel_multiplier=1)
    bqf = coordp.tile([128, 1], f32, name="bqf")
    nc.gpsimd.tensor_copy(out=bqf[:], in_=pq_i[:])
    nc.gpsimd.tensor_scalar_mul(bqf[:], bqf[:], float(LVLSUM))

    ident = const_pool.tile([128, 128], bf16, name="ident")
    make_identity(nc, ident[:])

    # weight / index tiles
    idx_i = coordp.tile([128, NBLK, NUM_LEVELS], i32, name="idx_i")
    wx0e = coordp.tile([128, NBLK, NUM_LEVELS, KK], bf16, name="wx0e")
    wx1e = coordp.tile([128, NBLK, NUM_LEVELS, KK], bf16, name="wx1e")
    wy0e = coordp.tile([128, NBLK, NUM_LEVELS, KK], bf16, name="wy0e")
    wy1e = coordp.tile([128, NBLK, NUM_LEVELS, KK], bf16, name="wy1e")

    sh3 = [128, NBLK, NUM_LEVELS]
    ixf = coordp.tile(sh3, f32, name="ixf")
    iyf = coordp.tile(sh3, f32, name="iyf")
    fxe = coordp.tile(sh3, f32, name="fxe")
    fye = coordp.tile(sh3, f32, name="fye")

    def emit_idx():
        """floor/frac for all levels + gather start indices (vector)."""
        for (src, ff, fr, nm) in ((cxs, ixf, fxe, "x"), (cys, iyf, fye, "y")):
            # xs = src / 2^l; floor via i32 cast (rounds on HW) + is_gt fix
            xs = small.tile(sh3, f32, name=f"xs_{nm}", tag="xs")
            nc.vector.tensor_tensor(
                xs[:],
                src[:].unsqueeze(2).to_broadcast(sh3),
                invv[:].unsqueeze(1).to_broadcast(sh3),
                op=Alu.mult,
            )
            ii = small.tile(sh3, i32, name=f"ii_{nm}", tag="ii")
            nc.vector.tensor_copy(out=ii[:], in_=xs[:])
            nc.vector.tensor_copy(out=ff[:], in_=ii[:])
            adj = small.tile(sh3, f32, name=f"adj_{nm}", tag="adj")
            nc.vector.tensor_tensor(adj[:], ff[:], xs[:], op=Alu.is_gt)
            nc.vector.tensor_tensor(ff[:], ff[:], adj[:], op=Alu.subtract)
            nc.vector.tensor_tensor(fr[:], xs[:], ff[:], op=Alu.subtract)

        # gather run start: PAD + p*LVLSUM + lvl_off + (iy-4)*Wl + (ix-4)
        t1 = small.tile(sh3, f32, name="t1", tag="t1")
        nc.vector.tensor_tensor(
            t1[:], iyf[:], wlv[:].unsqueeze(1).to_broadcast(sh3), op=Alu.mult
        )
        nc.vector.tensor_tensor(t1[:], t1[:], ixf[:], op=Alu.add)
        nc.vector.tensor_tensor(
            t1[:], t1[:], bqf[:].unsqueeze(2).to_broadcast(sh3), op=Alu.add
        )
        nc.vector.tensor_tensor(
            t1[:], t1[:], cvecv[:].unsqueeze(1).to_broadcast(sh3), op=Alu.add
        )
        nc.vector.tensor_copy(out=idx_i[:], in_=t1[:])  # exact ints

    def emit_weights():
        """Bilinear weights with OOB masks, all levels batched (vector)."""
        sh4 = [128, NBLK, NUM_LEVELS, PS]
        shk = [128, NBLK, NUM_LEVELS, KK]
        for (w0t, w1t, frac, posf, limv) in (
            (wx0e, wx1e, fxe, ixf, limxv),
            (wy0e, wy1e, fye, iyf, limyv),
        ):
            pos = small.tile(sh4, f32, name="pos", tag="pos")
            nc.vector.tensor_tensor(
                pos[:],
                posf[:].unsqueeze(3).to_broadcast(sh4),
                crampf[:].unsqueeze(1).unsqueeze(1).to_broadcast(sh4),
                op=Alu.add,
            )
            # in-bounds <=> |2*pos - lim| <= lim
            nc.vector.tensor_scalar_mul(pos[:], pos[:], 2.0)
            nc.vector.tensor_tensor(
                pos[:], pos[:],
                limv[:].unsqueeze(1).unsqueeze(3).to_broadcast(sh4),
                op=Alu.subtract,
            )
            ok = small.tile(sh4, f32, name="ok", tag="ok")
            nc.scalar.activation(ok[:], pos[:], mybir.ActivationFunctionType.Abs)
            nc.vector.tensor_tensor(
                ok[:], ok[:],
                limv[:].unsqueeze(1).unsqueeze(3).to_broadcast(sh4),
                op=Alu.is_le,
            )
            w0 = small.tile(sh3, f32, name="w0", tag="w0")
            nc.vector.tensor_scalar(w0[:], frac[:], -1.0, 1.0,
                                    op0=Alu.mult, op1=Alu.add)  # 1 - frac
            nc.vector.tensor_tensor(
                w0t[:], w0[:].unsqueeze(3).to_broadcast(shk),
                ok[:, :, :, 0:KK], op=Alu.mult,
            )
            nc.vector.tensor_tensor(
                w1t[:], frac[:].unsqueeze(3).to_broadcast(shk),
                ok[:, :, :, 1:PS], op=Alu.mult,
            )

    # ---------------- fmap2 pyramid pooling (two row-chains) ---------------
    # pooled levels keep raw SUMS; the 1/16 * 0.25^l scale is in the drain.
    # chain c covers output rows [c*Hl/2, (c+1)*Hl/2) of every level; chain 0
    # only needs f2 columns 0:4096 (sub-loads 0-1), chain 1 the rest.
    f2_lv = [f2_halves]
    for l in range(1, NUM_LEVELS):
        Wl, Hl = LVL_W[l], LVL_H[l]
        f2_lv.append(
            [f2_pool.tile([128, Hl * Wl], bf16, name=f"f2l{l}_{k}") for k in range(2)]
        )

    def emit_pool_chain(c):
        for l in range(1, NUM_LEVELS):
            Wl, Hl = LVL_W[l], LVL_H[l]
            pw, ph = LVL_W[l - 1], LVL_H[l - 1]
            r0, r1 = c * ph // 2, (c + 1) * ph // 2  # prev-level row range
            for k in range(2):
                prev = f2_lv[l - 1][k][:].rearrange(
                    "p (h w two) -> p h w two", h=ph, w=pw // 2, two=2
                )
                s1 = small.tile(
                    [128, ph // 2, pw // 2], bf16, name=f"s1_{c}_{l}_{k}",
                    tag="poolt", bufs=1,
                )
                nc.vector.tensor_tensor(
                    s1[:], prev[:, r0:r1, :, 0], prev[:, r0:r1, :, 1], op=Alu.add
                )
                s1v = s1[:].rearrange(
                    "p (h2 two) w -> p h2 two w", h2=Hl // 2, two=2
                )
                curv = f2_lv[l][k][:].rearrange("p (h w) -> p h w", h=Hl, w=Wl)
                nc.vector.tensor_tensor(
                    curv[:, c * Hl // 2 : (c + 1) * Hl // 2, :],
                    s1v[:, :, 0, :], s1v[:, :, 1, :], op=Alu.add,
                )

    # ---------------- patch tiles (gather destinations) --------------------
    patch = [
        patchp.tile([128, NBLK, ROWL[l]], bf16, name=f"patch{l}")
        for l in range(NUM_LEVELS)
    ]
    outq = patchp.tile([128, NBLK, NUM_LEVELS, KK, KK], bf16, name="outq")
    outq_v = outq[:].rearrange("p b l dy dx -> p b (l dy dx)")
    outT = [
        outp.tile([128, NBLK, 128], f32, name=f"outT{k}")
        for k in range(len(CHUNKS))
    ]

    # ---------------- GEMM helpers -----------------------------------------
    drain_parity = [0]

    def _drain(dst, pt, scale_l):
        if drain_parity[0] % 2 == 0:
            nc.vector.tensor_scalar_mul(dst[:], pt[:], scale_l)
        else:
            nc.scalar.mul(dst[:], pt[:], scale_l)
        drain_parity[0] += 1

    def _mm(pt_slice, b, l, k, n0, n1, start, stop):
        nc.tensor.matmul(
            pt_slice,
            f1h[k][:, b * 128 : (b + 1) * 128],
            f2_lv[l][k][:, n0:n1],
            start=start,
            stop=stop,
        )

    def emit_gemm_l0_range(b, cv_sb, c0, c1):
        """L0 GEMM for columns [c0, c1) in 1024-col PSUM groups."""
        for n0 in range(c0, c1, 2 * MM_N):
            pt = psum.tile([128, 2 * MM_N], f32, name="pt", tag="pt")
            for j, m0 in enumerate((n0, n0 + MM_N)):
                _mm(pt[:, j * MM_N : (j + 1) * MM_N], b, 0, 0, m0, m0 + MM_N,
                    True, False)
                _mm(pt[:, j * MM_N : (j + 1) * MM_N], b, 0, 1, m0, m0 + MM_N,
                    False, True)
            _drain(cv_sb[:, n0 : n0 + 2 * MM_N], pt, 1.0 / 16.0)

    def emit_gemm_l123(b, cv_sb):
        # L1: two 1024-col groups
        for n0 in range(0, LVL_N[1], 2 * MM_N):
            pt = psum.tile([128, 2 * MM_N], f32, name="pt", tag="pt")
            for j, m0 in enumerate((n0, n0 + MM_N)):
                _mm(pt[:, j * MM_N : (j + 1) * MM_N], b, 1, 0, m0, m0 + MM_N,
                    True, False)
                _mm(pt[:, j * MM_N : (j + 1) * MM_N], b, 1, 1, m0, m0 + MM_N,
                    False, True)
            _drain(
                cv_sb[:, LVL_OFF[1] + n0 : LVL_OFF[1] + n0 + 2 * MM_N],
                pt, (1.0 / 16.0) * 0.25,
            )
        # L2 (512) + L3 (128) share one 640-col tail tile; separate drains
        # because their scales differ
        ptw = psum.tile([128, 2 * MM_N], f32, name="pt", tag="pt")
        pt = ptw[:, 0 : LVL_N[2] + LVL_N[3]]
        _mm(pt[:, 0 : LVL_N[2]], b, 2, 0, 0, LVL_N[2], True, False)
        _mm(pt[:, 0 : LVL_N[2]], b, 2, 1, 0, LVL_N[2], False, True)
        _mm(pt[:, LVL_N[2] :], b, 3, 0, 0, LVL_N[3], True, False)
        _mm(pt[:, LVL_N[2] :], b, 3, 1, 0, LVL_N[3], False, True)
        _drain(cv_sb[:, LVL_OFF[2] : LVL_OFF[2] + LVL_N[2]],
               pt[:, 0 : LVL_N[2]], (1.0 / 16.0) * 0.25 ** 2)
        _drain(cv_sb[:, LVL_OFF[3] : LVL_OFF[3] + LVL_N[3]],
               pt[:, LVL_N[2] :], (1.0 / 16.0) * 0.25 ** 3)

    def emit_write_l0(b, cv_sb):
        nc.sync.dma_start(
            out=cv_dram[b][PAD : PAD + BQS].rearrange("(q s) -> q s", s=LVLSUM)[
                :, 0 : LVL_N[0]
            ],
            in_=cv_sb[:, 0 : LVL_N[0]],
        )

    def emit_write_l123(b, cv_sb):
        nc.sync.dma_start(
            out=cv_dram[b][PAD : PAD + BQS].rearrange("(q s) -> q s", s=LVLSUM)[
                :, LVL_N[0] : LVLSUM
            ],
            in_=cv_sb[:, LVL_N[0] : LVLSUM],
        )

    def emit_gather(b, lvls):
        # one indirect gather per level: contiguous run covering the patch
        cv2d = cv_dram[b][:].rearrange("(a x) -> a x", a=1024)
        for l in lvls:
            nc.gpsimd.indirect_dma_start(
                out=patch[l][:, b, 0 : RUN[l]],
                out_offset=None,
                in_=cv2d,
                in_offset=bass.IndirectOffsetOnAxis(
                    ap=idx_i[:, b, l].unsqueeze(1), axis=1
                ),
            )

    def emit_bilinear(b0, b1, lvls):
        """Bilinear for blocks [b0, b1) at the given levels (vector)."""
        nb = b1 - b0
        for l in lvls:
            Wl = LVL_W[l]
            Pv = patch[l][:].rearrange("p b (r c) -> p b r c", r=PS, c=Wl)
            bshape_x = [128, nb, PS, KK]
            tx = txp.tile([128, nb, PS, KK], bf16, name=f"tx{b0}{l}", tag="tx")
            tx2 = txp.tile([128, nb, PS, KK], bf16, name=f"tx2{b0}{l}", tag="tx2")
            nc.vector.tensor_tensor(
                tx[:], Pv[:, b0:b1, :, 0:KK],
                wx0e[:, b0:b1, l, :].unsqueeze(2).to_broadcast(bshape_x),
                op=Alu.mult,
            )
            nc.vector.tensor_tensor(
                tx2[:], Pv[:, b0:b1, :, 1:PS],
                wx1e[:, b0:b1, l, :].unsqueeze(2).to_broadcast(bshape_x),
                op=Alu.mult,
            )
            nc.vector.tensor_tensor(tx[:], tx[:], tx2[:], op=Alu.add)

            bshape_y = [128, nb, KK, KK]
            oq2 = txp.tile([128, nb, KK, KK], bf16, name=f"oq2{b0}{l}", tag="oq2")
            nc.vector.tensor_tensor(
                oq2[:], tx[:, :, 0:KK, :],
                wy0e[:, b0:b1, l, :].unsqueeze(3).to_broadcast(bshape_y),
                op=Alu.mult,
            )
            nc.vector.tensor_tensor(
                outq[:, b0:b1, l], tx[:, :, 1:PS, :],
                wy1e[:, b0:b1, l, :].unsqueeze(3).to_broadcast(bshape_y),
                op=Alu.mult,
            )
            nc.vector.tensor_tensor(
                outq[:, b0:b1, l], outq[:, b0:b1, l], oq2[:], op=Alu.add
            )

    def emit_transpose_out(b0, b1, dma=True):
        for k, (c0, nk) in enumerate(CHUNKS):
            for b in range(b0, b1):
                ptt = psum_t.tile([128, 128], bf16, name="ptt", tag="ptt")
                nc.tensor.transpose(
                    out=ptt[:nk, :], in_=outq_v[:, b, c0 : c0 + nk],
                    identity=ident[:],
                )
                if b % 2 == 0:
                    nc.vector.tensor_copy(out=outT[k][0:nk, b, :], in_=ptt[:nk, :])
                else:
                    nc.scalar.copy(out=outT[k][0:nk, b, :], in_=ptt[:nk, :])
        if dma:
            for k, (c0, nk) in enumerate(CHUNKS):
                nc.sync.dma_start(
                    out=out_ext[c0 : c0 + nk, b0:b1, :],
                    in_=outT[k][0:nk, b0:b1, :],
                )

    # ---------------- schedule ---------------------------------------------
    # Blocks 0-2's L0 GEMM interleaved by f2 sub-load so the 4.5MB input
    # stream is fully hidden; pooling chains and idx/weights calc slot into
    # the vector stream between the early drains.
    NEARLY = 3
    cv_sbs = [
        cvp.tile([128, LVLSUM], bf16, name=f"cv_sb{b}", tag="cv_sb")
        for b in range(NEARLY)
    ]
    for s in range(NSUB):
        for b in range(NEARLY):
            emit_gemm_l0_range(b, cv_sbs[b], s * SUBW, (s + 1) * SUBW)
        if s == 0:
            emit_pool_chain(0)             # gated on f2 sub-loads 0-1
            emit_idx()
        elif s == 1:
            emit_pool_chain(1)             # gated on f2 sub-loads 2-3
    emit_weights()
    for b in range(NEARLY):
        emit_write_l0(b, cv_sbs[b])
        emit_gemm_l123(b, cv_sbs[b])
        emit_write_l123(b, cv_sbs[b])
        emit_gather(b, [0, 1, 2, 3])
    for b in range(NEARLY, NBLK):
        cv_sb = cvp.tile([128, LVLSUM], bf16, name=f"cv_sb{b}", tag="cv_sb")
        emit_gemm_l0_range(b, cv_sb, 0, LVL_N[0])
        emit_write_l0(b, cv_sb)
        emit_gemm_l123(b, cv_sb)
        emit_write_l123(b, cv_sb)
        emit_gather(b, [0, 1, 2, 3])
        if b == 4:
            emit_bilinear(0, 2, [0, 1, 2, 3])
        elif b == 5:
            emit_bilinear(2, 4, [0, 1, 2, 3])
            emit_transpose_out(0, 2)
        elif b == 6:
            emit_transpose_out(2, 4)
            emit_bilinear(4, 6, [0, 1, 2, 3])
    emit_transpose_out(4, 6)
    emit_bilinear(6, 8, [0, 1, 2, 3])
    emit_transpose_out(6, 8)

    if dbg is not None:
        nc.sync.dma_start(
            out=dbg["idx"][:], in_=idx_i[:].rearrange("p b l -> p (b l)")
        )
        nc.sync.dma_start(
            out=dbg["patch0"][:], in_=patch[0][:].rearrange("p b r -> p (b r)")
        )
        nc.sync.dma_start(
            out=dbg["patch3"][:], in_=patch[3][:].rearrange("p b r -> p (b r)")
        )
        nc.sync.dma_start(
            out=dbg["wx0"][:], in_=wx0e[:].rearrange("p b l k -> p (b l k)")
        )
        nc.sync.dma_start(
            out=dbg["cv0"][:],
            in_=cv_dram[0][0 : PAD + 2 * LVLSUM].rearrange("(p x) -> p x", p=128),
        )
        nc.sync.dma_start(
            out=dbg["outq"][:], in_=outq[:].rearrange("p b l dy dx -> p (b l dy dx)")
        )


def build_program(debug=False):
    """Build (once) the single-core SPMD bass program."""
    key = ("nc", debug)
    if key in _CACHE:
        return _CACHE[key]
    import concourse.tile as tile
    import concourse.mybir as mybir
    from concourse import bacc

    f32 = mybir.dt.float32
    bf16 = mybir.dt.bfloat16
    i32 = mybir.dt.int32
    nc = bacc.Bacc(
        "TRN2",
        target_bir_lowering=False,
        debug=False,
        enable_asserts=True,
        num_devices=NCORES,
    )
    f1c = nc.dram_tensor("f1c", [D, QPC], bf16, kind="ExternalInput").ap()
    f2 = nc.dram_tensor("f2", [D, H * W], bf16, kind="ExternalInput").ap()
    crd = nc.dram_tensor("crd", [2, QPC], f32, kind="ExternalInput").ap()
    out = nc.dram_tensor("out", [NCH, H // NCORES, W], f32, kind="ExternalOutput").ap()
    dbg = None
    if debug:
        dbg = {
            "idx": nc.dram_tensor(
                "dbg_idx", [128, NBLK * NUM_LEVELS], i32, kind="ExternalOutput"
            ).ap(),
            "patch0": nc.dram_tensor(
                "dbg_patch0", [128, NBLK * ROWL[0]], bf16, kind="ExternalOutput"
            ).ap(),
            "patch3": nc.dram_tensor(
                "dbg_patch3", [128, NBLK * ROWL[3]], bf16, kind="ExternalOutput"
            ).ap(),
            "wx0": nc.dram_tensor(
                "dbg_wx0", [128, NBLK * NUM_LEVELS * KK], bf16, kind="ExternalOutput"
            ).ap(),
            "cv0": nc.dram_tensor(
                "dbg_cv0", [128, (PAD + 2 * LVLSUM) // 128], bf16,
                kind="ExternalOutput",
            ).ap(),
            "outq": nc.dram_tensor(
                "dbg_outq", [128, NBLK * NCH], bf16, kind="ExternalOutput"
            ).ap(),
        }

    from contextlib import ExitStack

    with tile.TileContext(nc) as tc, ExitStack() as ctx:
        _emit(ctx, tc, out, f1c, f2, crd, dbg=dbg)
    nc.compile()
    _CACHE[key] = nc
    return nc


def make_in_maps(fmap1, fmap2, coords):
    import ml_dtypes

    bf = ml_dtypes.bfloat16
    f1 = np.ascontiguousarray(
        np.asarray(fmap1, dtype=np.float32).reshape(D, H * W)
    ).astype(bf)
    f2 = np.ascontiguousarray(
        np.asarray(fmap2, dtype=np.float32).reshape(D, H * W)
    ).astype(bf)
    crd = np.asarray(coords, dtype=np.float32).reshape(2, H * W)
    in_maps = []
    for c in range(NCORES):
        sl = slice(c * QPC, (c + 1) * QPC)
        in_maps.append(
            {
                "f1c": np.ascontiguousarray(f1[:, sl]),
                "f2": f2,
                "crd": np.ascontiguousarray(crd[:, sl]),
            }
        )
    return in_maps


def kernel(fmap1, fmap2, coords):
    from concourse.bass_utils import run_bass_kernel_spmd

    nc = build_program()
    in_maps = make_in_maps(fmap1, fmap2, coords)
    res = run_bass_kernel_spmd(nc, in_maps, list(range(NCORES)))
    parts = [res.results[c]["out"] for c in range(NCORES)]  # [324, 8, 128] each
    full = np.concatenate(parts, axis=1)  # [324, 64, 128]
    return full[None].astype(np.float32)


# revision 18
# speedup vs baseline: 1.2364x; 1.0750x over previous
# Correlation2D (RAFT-style correlation pyramid lookup) on 8 TRN2 NeuronCores.
#
# Sharding: data-parallel over the bs*h*w query axis. Each core owns 1024
# queries (= 8 image rows). Per block of 128 queries it computes its slice of
# the cost volume via a bf16 GEMM (fmap2 replicated, pooling folded into
# fmap2), writes the 4-level pyramid per-query-contiguous to DRAM (bf16), and
# gathers ONE contiguous run per (block, level) spanning the whole 10x10
# patch (rows are Wl apart inside the run; the bilinear stage reads the run
# through a strided view). Bilinear combine is separable in bf16, spread
# across the GEMM pipeline; output is PE-transposed to channel-major.
# Output per core is [324, 8, 128] f32, host concatenates along y.
#
# Scheduling notes (engine streams are in-order, so emission order matters):
# - f2 loads in 4 column sub-chunks; block0/block1 level-0 GEMMs run while
#   the rest of f2 streams in; f2 pooling in two row-chains gated on subloads.
# - matmuls grouped 4 N-chunks per K-half so walrus can reuse LDWEIGHTS.
# - PSUM drains alternate vector/scalar; weights/idx calc sits between
#   early drains; bilinear is emitted 2 levels at a time after gathers.
import numpy as np

# ---- problem constants (hardcoded per contest contract) ----
H, W = 64, 128
D = 256
NUM_LEVELS = 4
RADIUS = 4
KK = 2 * RADIUS + 1        # 9
PS = KK + 1                # 10x10 patch per (query, level)
NCORES = 8
QPC = (H * W) // NCORES    # 1024 queries per core
NBLK = QPC // 128          # 8 blocks of 128 queries
LVL_W = [W >> l for l in range(NUM_LEVELS)]            # 128 64 32 16
LVL_H = [H >> l for l in range(NUM_LEVELS)]            # 64 32 16 8
LVL_N = [LVL_W[l] * LVL_H[l] for l in range(NUM_LEVELS)]   # 8192 2048 512 128
LVL_OFF = [sum(LVL_N[:l]) for l in range(NUM_LEVELS)]  # 0 8192 10240 10752
LVLSUM = sum(LVL_N)        # 10880
RUN = [KK * LVL_W[l] + PS for l in range(NUM_LEVELS)]  # 1162 586 298 154
ROWL = [PS * LVL_W[l] for l in range(NUM_LEVELS)]      # 1280 640 320 160
PAD = 1024                 # zeroed head/tail pad (elements) per block buffer
BQS = 128 * LVLSUM         # elements of cv per block
BTOT = PAD + BQS + PAD     # per-block DRAM tensor elements (bf16)
NCH = NUM_LEVELS * KK * KK  # 324 output channels
MM_N = 512                 # matmul N-chunk (one PSUM bank of f32)
GRP = 4                    # N-chunks per weight-load group
CHUNKS = [(0, 128), (128, 128), (256, NCH - 256)]  # output channel chunks

_CACHE = {}


def _emit(ctx, tc, out_ext, f1c, f2, crd, dbg=None):
    import concourse.bass as bass
    import concourse.mybir as mybir
    from concourse.masks import make_identity

    nc = tc.nc
    f32 = mybir.dt.float32
    bf16 = mybir.dt.bfloat16
    i32 = mybir.dt.int32
    Alu = mybir.AluOpType

    const_pool = ctx.enter_context(tc.tile_pool(name="constp", bufs=1))
    f2_pool = ctx.enter_context(tc.tile_pool(name="f2p", bufs=1))
    f1_pool = ctx.enter_context(tc.tile_pool(name="f1p", bufs=1))
    coordp = ctx.enter_context(tc.tile_pool(name="coordp", bufs=1))
    small = ctx.enter_context(tc.tile_pool(name="small", bufs=2))
    cvp = ctx.enter_context(tc.tile_pool(name="cvp", bufs=3))
    patchp = ctx.enter_context(tc.tile_pool(name="patchp", bufs=1))
    txp = ctx.enter_context(tc.tile_pool(name="txp", bufs=2))
    outp = ctx.enter_context(tc.tile_pool(name="outp", bufs=1))
    # pt1024 spans 2 PSUM banks (matmuls stay within one bank each); the
    # L2+L3 tail reuses the same slots (512 in bank a, 128 in bank b)
    psum = ctx.enter_context(tc.tile_pool(name="psum", bufs=3, space="PSUM"))
    psum_t = ctx.enter_context(tc.tile_pool(name="psumt", bufs=2, space="PSUM"))
    dramp = ctx.enter_context(tc.tile_pool(name="dramp", bufs=1, space="DRAM"))

    # ------------- per-block DRAM cv buffers (query-contiguous pyramid) ----
    cv_dram = [dramp.tile([BTOT], bf16, name=f"cv_dram{b}") for b in range(NBLK)]

    # ---------------- input loads (ordered: small/critical first) ----------
    cxs = coordp.tile([128, NBLK], f32, name="cxs")
    cys = coordp.tile([128, NBLK], f32, name="cys")
    nc.scalar.dma_start(out=cxs[:], in_=crd[0, :].rearrange("(b p) -> p b", p=128))
    nc.scalar.dma_start(out=cys[:], in_=crd[1, :].rearrange("(b p) -> p b", p=128))

    f1h = []
    for k in range(2):
        t = f1_pool.tile([128, QPC], bf16, name=f"f1h{k}")
        nc.sync.dma_start(out=t[:], in_=f1c[k * 128 : (k + 1) * 128, :])
        f1h.append(t)

    # f2 halves in column sub-loads (small first) so GEMM starts early
    SUBS = [1024, 1024, 2048, 2048, 2048]
    SUBB = [sum(SUBS[:i]) for i in range(len(SUBS) + 1)]  # boundaries
    f2_halves = []
    for k in range(2):
        f2h = f2_pool.tile([128, LVL_N[0]], bf16, name=f"f2h{k}")
        f2_halves.append(f2h)
    for s in range(len(SUBS)):
        for k in range(2):
            nc.sync.dma_start(
                out=f2_halves[k][:, SUBB[s] : SUBB[s + 1]],
                in_=f2[k * 128 : (k + 1) * 128, SUBB[s] : SUBB[s + 1]],
            )

    # zero head/tail pads (gather runs can poke into them; must stay finite)
    ztile = const_pool.tile([128, 8], bf16, name="ztile")
    nc.gpsimd.memset(ztile[:], 0.0)
    for b in range(NBLK):
        nc.sync.dma_start(
            out=cv_dram[b][0:PAD].rearrange("(p x) -> p x", p=128), in_=ztile[:]
        )
        nc.sync.dma_start(
            out=cv_dram[b][PAD + BQS : BTOT].rearrange("(p x) -> p x", p=128),
            in_=ztile[:],
        )

    # ---------------- constants -------------------------------------------
    # per-level vectors [128, 4] (exact values via memset per lane)
    invv = const_pool.tile([128, NUM_LEVELS], f32, name="invv")
    wlv = const_pool.tile([128, NUM_LEVELS], f32, name="wlv")
    limxv = const_pool.tile([128, NUM_LEVELS], f32, name="limxv")
    limyv = const_pool.tile([128, NUM_LEVELS], f32, name="limyv")
    cvecv = const_pool.tile([128, NUM_LEVELS], f32, name="cvecv")
    for l in range(NUM_LEVELS):
        nc.gpsimd.memset(invv[:, l : l + 1], 1.0 / (1 << l))
        nc.gpsimd.memset(wlv[:, l : l + 1], float(LVL_W[l]))
        nc.gpsimd.memset(limxv[:, l : l + 1], float(LVL_W[l] - 1))
        nc.gpsimd.memset(limyv[:, l : l + 1], float(LVL_H[l] - 1))
        nc.gpsimd.memset(
            cvecv[:, l : l + 1],
            float(PAD + LVL_OFF[l] - RADIUS * LVL_W[l] - RADIUS),
        )

    # c ramp: -4..5 (patch-col -> absolute offset from floor(coord))
    cramp_i = const_pool.tile([128, PS], i32, name="cramp_i")
    nc.gpsimd.iota(cramp_i[:], pattern=[[1, PS]], base=-RADIUS, channel_multiplier=0)
    crampf = const_pool.tile([128, PS], f32, name="crampf")
    nc.gpsimd.tensor_copy(out=crampf[:], in_=cramp_i[:])

    # per-query element base offset of its pyramid inside its block's buffer
    pq_i = coordp.tile([128, 1], i32, name="pq_i")
    nc.gpsimd.iota(pq_i[:], pattern=[[1, 1]], base=0, channel_multiplier=1)
    bqf = coordp.tile([128, 1], f32, name="bqf")
    nc.gpsimd.tensor_copy(out=bqf[:], in_=pq_i[:])
    nc.gpsimd.tensor_scalar_mul(bqf[:], bqf[:], float(LVLSUM))

    ident = const_pool.tile([128, 128], bf16, name="ident")
    make_identity(nc, ident[:])

    # weight / index tiles
    idx_i = coordp.tile([128, NBLK, NUM_LEVELS], i32, name="idx_i")
    wx0e = coordp.tile([128, NBLK, NUM_LEVELS, KK], bf16, name="wx0e")
    wx1e = coordp.tile([128, NBLK, NUM_LEVELS, KK], bf16, name="wx1e")
    wy0e = coordp.tile([128, NBLK, NUM_LEVELS, KK], bf16, name="wy0e")
    wy1e = coordp.tile([128, NBLK, NUM_LEVELS, KK], bf16, name="wy1e")

    sh3 = [128, NBLK, NUM_LEVELS]
    ixf = coordp.tile(sh3, f32, name="ixf")
    iyf = coordp.tile(sh3, f32, name="iyf")
    fxe = coordp.tile(sh3, f32, name="fxe")
    fye = coordp.tile(sh3, f32, name="fye")

    def emit_idx():
        """floor/frac for all levels + gather start indices (vector)."""
        for (src, ff, fr, nm) in ((cxs, ixf, fxe, "x"), (cys, iyf, fye, "y")):
            # xs = src / 2^l; floor via i32 cast (rounds on HW) + is_gt fix
            xs = small.tile(sh3, f32, name=f"xs_{nm}", tag="xs")
            nc.vector.tensor_tensor(
                xs[:],
                src[:].unsqueeze(2).to_broadcast(sh3),
                invv[:].unsqueeze(1).to_broadcast(sh3),
                op=Alu.mult,
            )
            ii = small.tile(sh3, i32, name=f"ii_{nm}", tag="ii")
            nc.vector.tensor_copy(out=ii[:], in_=xs[:])
            nc.vector.tensor_copy(out=ff[:], in_=ii[:])
            adj = small.tile(sh3, f32, name=f"adj_{nm}", tag="adj")
            nc.vector.tensor_tensor(adj[:], ff[:], xs[:], op=Alu.is_gt)
            nc.vector.tensor_tensor(ff[:], ff[:], adj[:], op=Alu.subtract)
            nc.vector.tensor_tensor(fr[:], xs[:], ff[:], op=Alu.subtract)

        # gather run start: PAD + p*LVLSUM + lvl_off + (iy-4)*Wl + (ix-4)
        t1 = small.tile(sh3, f32, name="t1", tag="t1")
        nc.vector.tensor_tensor(
            t1[:], iyf[:], wlv[:].unsqueeze(1).to_broadcast(sh3), op=Alu.mult
        )
        nc.vector.tensor_tensor(t1[:], t1[:], ixf[:], op=Alu.add)
        nc.vector.tensor_tensor(
            t1[:], t1[:], bqf[:].unsqueeze(2).to_broadcast(sh3), op=Alu.add
        )
        nc.vector.tensor_tensor(
            t1[:], t1[:], cvecv[:].unsqueeze(1).to_broadcast(sh3), op=Alu.add
        )
        nc.vector.tensor_copy(out=idx_i[:], in_=t1[:])  # exact ints

    def emit_weights():
        """Bilinear weights with OOB masks, all levels batched (vector)."""
        sh4 = [128, NBLK, NUM_LEVELS, PS]
        shk = [128, NBLK, NUM_LEVELS, KK]
        for (w0t, w1t, frac, posf, limv) in (
            (wx0e, wx1e, fxe, ixf, limxv),
            (wy0e, wy1e, fye, iyf, limyv),
        ):
            pos = small.tile(sh4, f32, name="pos", tag="pos")
            nc.vector.tensor_tensor(
                pos[:],
                posf[:].unsqueeze(3).to_broadcast(sh4),
                crampf[:].unsqueeze(1).unsqueeze(1).to_broadcast(sh4),
                op=Alu.add,
            )
            # in-bounds <=> |2*pos - lim| <= lim
            nc.vector.tensor_scalar_mul(pos[:], pos[:], 2.0)
            nc.vector.tensor_tensor(
                pos[:], pos[:],
                limv[:].unsqueeze(1).unsqueeze(3).to_broadcast(sh4),
                op=Alu.subtract,
            )
            ok = small.tile(sh4, f32, name="ok", tag="ok")
            nc.scalar.activation(ok[:], pos[:], mybir.ActivationFunctionType.Abs)
            nc.vector.tensor_tensor(
                ok[:], ok[:],
                limv[:].unsqueeze(1).unsqueeze(3).to_broadcast(sh4),
                op=Alu.is_le,
            )
            w0 = small.tile(sh3, f32, name="w0", tag="w0")
            nc.vector.tensor_scalar(w0[:], frac[:], -1.0, 1.0,
                                    op0=Alu.mult, op1=Alu.add)  # 1 - frac
            nc.vector.tensor_tensor(
                w0t[:], w0[:].unsqueeze(3).to_broadcast(shk),
                ok[:, :, :, 0:KK], op=Alu.mult,
            )
            nc.vector.tensor_tensor(
                w1t[:], frac[:].unsqueeze(3).to_broadcast(shk),
                ok[:, :, :, 1:PS], op=Alu.mult,
            )

    # ---------------- fmap2 pyramid pooling (two row-chains) ---------------
    # pooled levels keep raw SUMS; the 1/16 * 0.25^l scale is in the drain.
    # chain c covers output rows [c*Hl/2, (c+1)*Hl/2) of every level; chain 0
    # only needs f2 columns 0:4096 (sub-loads 0-1), chain 1 the rest.
    f2_lv = [f2_halves]
    for l in range(1, NUM_LEVELS):
        Wl, Hl = LVL_W[l], LVL_H[l]
        f2_lv.append(
            [f2_pool.tile([128, Hl * Wl], bf16, name=f"f2l{l}_{k}") for k in range(2)]
        )

    def emit_pool_chain(c):
        for l in range(1, NUM_LEVELS):
            Wl, Hl = LVL_W[l], LVL_H[l]
            pw, ph = LVL_W[l - 1], LVL_H[l - 1]
            r0, r1 = c * ph // 2, (c + 1) * ph // 2  # prev-level row range
            for k in range(2):
                prev = f2_lv[l - 1][k][:].rearrange(
                    "p (h w two) -> p h w two", h=ph, w=pw // 2, two=2
                )
                s1 = small.tile(
                    [128, ph // 2, pw // 2], bf16, name=f"s1_{c}_{l}_{k}",
                    tag="poolt", bufs=1,
                )
                nc.vector.tensor_tensor(
                    s1[:], prev[:, r0:r1, :, 0], prev[:, r0:r1, :, 1], op=Alu.add
                )
                s1v = s1[:].rearrange(
                    "p (h2 two) w -> p h2 two w", h2=Hl // 2, two=2
                )
                curv = f2_lv[l][k][:].rearrange("p (h w) -> p h w", h=Hl, w=Wl)
                nc.vector.tensor_tensor(
                    curv[:, c * Hl // 2 : (c + 1) * Hl // 2, :],
                    s1v[:, :, 0, :], s1v[:, :, 1, :], op=Alu.add,
                )

    # ---------------- patch tiles (gather destinations) --------------------
    patch = [
        patchp.tile([128, NBLK, ROWL[l]], bf16, name=f"patch{l}")
        for l in range(NUM_LEVELS)
    ]
    outq = patchp.tile([128, NBLK, NUM_LEVELS, KK, KK], bf16, name="outq")
    outq_v = outq[:].rearrange("p b l dy dx -> p b (l dy dx)")
    outT = [
        outp.tile([128, NBLK, 128], f32, name=f"outT{k}")
        for k in range(len(CHUNKS))
    ]

    # ---------------- GEMM helpers -----------------------------------------
    drain_parity = [0]

    VEC_DRAINS = {0, 3, 5, 8}  # 4 of 10 on vector, rest on scalar

    def _drain(dst, pt, scale_l):
        if drain_parity[0] % 10 in VEC_DRAINS:
            nc.vector.tensor_scalar_mul(dst[:], pt[:], scale_l)
        else:
            nc.scalar.mul(dst[:], pt[:], scale_l)
        drain_parity[0] += 1

    def _drain_scalar(dst, pt, scale_l):
        nc.scalar.mul(dst[:], pt[:], scale_l)

    def _mm(pt_slice, b, l, k, n0, n1, start, stop):
        nc.tensor.matmul(
            pt_slice,
            f1h[k][:, b * 128 : (b + 1) * 128],
            f2_lv[l][k][:, n0:n1],
            start=start,
            stop=stop,
        )

    def emit_gemm_l0_range(b, cv_sb, c0, c1):
        """L0 GEMM for columns [c0, c1) in 1024-col PSUM groups."""
        for n0 in range(c0, c1, 2 * MM_N):
            pt = psum.tile([128, 2 * MM_N], f32, name="pt", tag="pt")
            for j, m0 in enumerate((n0, n0 + MM_N)):
                _mm(pt[:, j * MM_N : (j + 1) * MM_N], b, 0, 0, m0, m0 + MM_N,
                    True, False)
                _mm(pt[:, j * MM_N : (j + 1) * MM_N], b, 0, 1, m0, m0 + MM_N,
                    False, True)
            _drain(cv_sb[:, n0 : n0 + 2 * MM_N], pt, 1.0 / 16.0)

    def emit_gemm_l123(b, cv_sb):
        # L1: two 1024-col groups
        for n0 in range(0, LVL_N[1], 2 * MM_N):
            pt = psum.tile([128, 2 * MM_N], f32, name="pt", tag="pt")
            for j, m0 in enumerate((n0, n0 + MM_N)):
                _mm(pt[:, j * MM_N : (j + 1) * MM_N], b, 1, 0, m0, m0 + MM_N,
                    True, False)
                _mm(pt[:, j * MM_N : (j + 1) * MM_N], b, 1, 1, m0, m0 + MM_N,
                    False, True)
            _drain(
                cv_sb[:, LVL_OFF[1] + n0 : LVL_OFF[1] + n0 + 2 * MM_N],
                pt, (1.0 / 16.0) * 0.25,
            )
        # L2 (512) + L3 (128) share one 640-col tail tile; separate drains
        # because their scales differ
        ptw = psum.tile([128, 2 * MM_N], f32, name="pt", tag="pt")
        pt = ptw[:, 0 : LVL_N[2] + LVL_N[3]]
        _mm(pt[:, 0 : LVL_N[2]], b, 2, 0, 0, LVL_N[2], True, False)
        _mm(pt[:, 0 : LVL_N[2]], b, 2, 1, 0, LVL_N[2], False, True)
        _mm(pt[:, LVL_N[2] :], b, 3, 0, 0, LVL_N[3], True, False)
        _mm(pt[:, LVL_N[2] :], b, 3, 1, 0, LVL_N[3], False, True)
        _drain_scalar(cv_sb[:, LVL_OFF[2] : LVL_OFF[2] + LVL_N[2]],
                      pt[:, 0 : LVL_N[2]], (1.0 / 16.0) * 0.25 ** 2)
        _drain_scalar(cv_sb[:, LVL_OFF[3] : LVL_OFF[3] + LVL_N[3]],
                      pt[:, LVL_N[2] :], (1.0 / 16.0) * 0.25 ** 3)

    def emit_write_l0(b, cv_sb):
        nc.sync.dma_start(
            out=cv_dram[b][PAD : PAD + BQS].rearrange("(q s) -> q s", s=LVLSUM)[
                :, 0 : LVL_N[0]
            ],
            in_=cv_sb[:, 0 : LVL_N[0]],
        )

    def emit_write_l123(b, cv_sb):
        nc.sync.dma_start(
            out=cv_dram[b][PAD : PAD + BQS].rearrange("(q s) -> q s", s=LVLSUM)[
                :, LVL_N[0] : LVLSUM
            ],
            in_=cv_sb[:, LVL_N[0] : LVLSUM],
        )

    def emit_gather(b, lvls):
        # one indirect gather per level: contiguous run covering the patch
        cv2d = cv_dram[b][:].rearrange("(a x) -> a x", a=1024)
        for l in lvls:
            nc.gpsimd.indirect_dma_start(
                out=patch[l][:, b, 0 : RUN[l]],
                out_offset=None,
                in_=cv2d,
                in_offset=bass.IndirectOffsetOnAxis(
                    ap=idx_i[:, b, l].unsqueeze(1), axis=1
                ),
            )

    def emit_bilinear(b0, b1, lvls, eng=None):
        """Bilinear for blocks [b0, b1) at the given levels."""
        eng = eng or nc.vector
        nb = b1 - b0
        for l in lvls:
            Wl = LVL_W[l]
            Pv = patch[l][:].rearrange("p b (r c) -> p b r c", r=PS, c=Wl)
            bshape_x = [128, nb, PS, KK]
            tx = txp.tile([128, nb, PS, KK], bf16, name=f"tx{b0}{l}", tag="tx")
            tx2 = txp.tile([128, nb, PS, KK], bf16, name=f"tx2{b0}{l}", tag="tx2")
            eng.tensor_tensor(
                tx[:], Pv[:, b0:b1, :, 0:KK],
                wx0e[:, b0:b1, l, :].unsqueeze(2).to_broadcast(bshape_x),
                op=Alu.mult,
            )
            eng.tensor_tensor(
                tx2[:], Pv[:, b0:b1, :, 1:PS],
                wx1e[:, b0:b1, l, :].unsqueeze(2).to_broadcast(bshape_x),
                op=Alu.mult,
            )
            eng.tensor_tensor(tx[:], tx[:], tx2[:], op=Alu.add)

            bshape_y = [128, nb, KK, KK]
            oq2 = txp.tile([128, nb, KK, KK], bf16, name=f"oq2{b0}{l}", tag="oq2")
            eng.tensor_tensor(
                oq2[:], tx[:, :, 0:KK, :],
                wy0e[:, b0:b1, l, :].unsqueeze(3).to_broadcast(bshape_y),
                op=Alu.mult,
            )
            eng.tensor_tensor(
                outq[:, b0:b1, l], tx[:, :, 1:PS, :],
                wy1e[:, b0:b1, l, :].unsqueeze(3).to_broadcast(bshape_y),
                op=Alu.mult,
            )
            eng.tensor_tensor(
                outq[:, b0:b1, l], outq[:, b0:b1, l], oq2[:], op=Alu.add
            )

    def emit_transpose_out(b0, b1, dma=True):
        for k, (c0, nk) in enumerate(CHUNKS):
            for b in range(b0, b1):
                ptt = psum_t.tile([128, 128], bf16, name="ptt", tag="ptt")
                nc.tensor.transpose(
                    out=ptt[:nk, :], in_=outq_v[:, b, c0 : c0 + nk],
                    identity=ident[:],
                )
                if b % 2 == 0:
                    nc.vector.tensor_copy(out=outT[k][0:nk, b, :], in_=ptt[:nk, :])
                else:
                    nc.scalar.copy(out=outT[k][0:nk, b, :], in_=ptt[:nk, :])
        if dma:
            for k, (c0, nk) in enumerate(CHUNKS):
                nc.sync.dma_start(
                    out=out_ext[c0 : c0 + nk, b0:b1, :],
                    in_=outT[k][0:nk, b0:b1, :],
                )

    # ---------------- schedule ---------------------------------------------
    # Blocks 0-2's L0 GEMM interleaved by f2 sub-load so the 4.5MB input
    # stream is fully hidden; pooling chains and idx/weights calc slot into
    # the vector stream between the early drains.
    NEARLY = 3
    cv_sbs = [
        cvp.tile([128, LVLSUM], bf16, name=f"cv_sb{b}", tag="cv_sb")
        for b in range(NEARLY)
    ]
    for s in range(len(SUBS)):
        for b in range(NEARLY):
            emit_gemm_l0_range(b, cv_sbs[b], SUBB[s], SUBB[s + 1])
        if s == 0:
            emit_idx()                     # needs only coords
        elif s == 2:
            emit_pool_chain(0)             # gated on f2 sub-loads 0-2
        elif s == 3:
            emit_pool_chain(1)             # gated on f2 sub-loads 3-4
    emit_weights()
    for b in range(NEARLY):
        emit_write_l0(b, cv_sbs[b])
        emit_gemm_l123(b, cv_sbs[b])
        emit_write_l123(b, cv_sbs[b])
        emit_gather(b, [0, 1, 2, 3])
    for b in range(NEARLY, NBLK):
        cv_sb = cvp.tile([128, LVLSUM], bf16, name=f"cv_sb{b}", tag="cv_sb")
        emit_gemm_l0_range(b, cv_sb, 0, LVL_N[0])
        emit_write_l0(b, cv_sb)
        emit_gemm_l123(b, cv_sb)
        emit_write_l123(b, cv_sb)
        emit_gather(b, [0, 1, 2, 3])
        if b == 5:
            emit_bilinear(0, 4, [0, 1], nc.vector)
            emit_bilinear(0, 4, [2, 3], nc.gpsimd)
        elif b == 6:
            emit_transpose_out(0, 4)
        elif b == 7:
            emit_bilinear(4, 6, [0, 1], nc.vector)
            emit_bilinear(4, 6, [2, 3], nc.gpsimd)
    emit_transpose_out(4, 6)
    emit_bilinear(6, 8, [0, 1], nc.vector)
    emit_bilinear(6, 8, [2, 3], nc.gpsimd)
    emit_transpose_out(6, 8)

    if dbg is not None:
        nc.sync.dma_start(
            out=dbg["idx"][:], in_=idx_i[:].rearrange("p b l -> p (b l)")
        )
        nc.sync.dma_start(
            out=dbg["patch0"][:], in_=patch[0][:].rearrange("p b r -> p (b r)")
        )
        nc.sync.dma_start(
            out=dbg["patch3"][:], in_=patch[3][:].rearrange("p b r -> p (b r)")
        )
        nc.sync.dma_start(
            out=dbg["wx0"][:], in_=wx0e[:].rearrange("p b l k -> p (b l k)")
        )
        nc.sync.dma_start(
            out=dbg["cv0"][:],
            in_=cv_dram[0][0 : PAD + 2 * LVLSUM].rearrange("(p x) -> p x", p=128),
        )
        nc.sync.dma_start(
            out=dbg["outq"][:], in_=outq[:].rearrange("p b l dy dx -> p (b l dy dx)")
        )


def build_program(debug=False):
    """Build (once) the single-core SPMD bass program."""
    key = ("nc", debug)
    if key in _CACHE:
        return _CACHE[key]
    import concourse.tile as tile
    import concourse.mybir as mybir
    from concourse import bacc

    f32 = mybir.dt.float32
    bf16 = mybir.dt.bfloat16
    i32 = mybir.dt.int32
    nc = bacc.Bacc(
        "TRN2",
        target_bir_lowering=False,
        debug=False,
        enable_asserts=True,
        num_devices=NCORES,
    )
    f1c = nc.dram_tensor("f1c", [D, QPC], bf16, kind="ExternalInput").ap()
    f2 = nc.dram_tensor("f2", [D, H * W], bf16, kind="ExternalInput").ap()
    crd = nc.dram_tensor("crd", [2, QPC], f32, kind="ExternalInput").ap()
    out = nc.dram_tensor("out", [NCH, H // NCORES, W], f32, kind="ExternalOutput").ap()
    dbg = None
    if debug:
        dbg = {
            "idx": nc.dram_tensor(
                "dbg_idx", [128, NBLK * NUM_LEVELS], i32, kind="ExternalOutput"
            ).ap(),
            "patch0": nc.dram_tensor(
                "dbg_patch0", [128, NBLK * ROWL[0]], bf16, kind="ExternalOutput"
            ).ap(),
            "patch3": nc.dram_tensor(
                "dbg_patch3", [128, NBLK * ROWL[3]], bf16, kind="ExternalOutput"
            ).ap(),
            "wx0": nc.dram_tensor(
                "dbg_wx0", [128, NBLK * NUM_LEVELS * KK], bf16, kind="ExternalOutput"
            ).ap(),
            "cv0": nc.dram_tensor(
                "dbg_cv0", [128, (PAD + 2 * LVLSUM) // 128], bf16,
                kind="ExternalOutput",
            ).ap(),
            "outq": nc.dram_tensor(
                "dbg_outq", [128, NBLK * NCH], bf16, kind="ExternalOutput"
            ).ap(),
        }

    from contextlib import ExitStack

    with tile.TileContext(nc) as tc, ExitStack() as ctx:
        _emit(ctx, tc, out, f1c, f2, crd, dbg=dbg)
    nc.compile()
    _CACHE[key] = nc
    return nc


def make_in_maps(fmap1, fmap2, coords):
    import ml_dtypes

    bf = ml_dtypes.bfloat16
    f1 = np.ascontiguousarray(
        np.asarray(fmap1, dtype=np.float32).reshape(D, H * W)
    ).astype(bf)
    f2 = np.ascontiguousarray(
        np.asarray(fmap2, dtype=np.float32).reshape(D, H * W)
    ).astype(bf)
    crd = np.asarray(coords, dtype=np.float32).reshape(2, H * W)
    in_maps = []
    for c in range(NCORES):
        sl = slice(c * QPC, (c + 1) * QPC)
        in_maps.append(
            {
                "f1c": np.ascontiguousarray(f1[:, sl]),
                "f2": f2,
                "crd": np.ascontiguousarray(crd[:, sl]),
            }
        )
    return in_maps


def kernel(fmap1, fmap2, coords):
    from concourse.bass_utils import run_bass_kernel_spmd

    nc = build_program()
    in_maps = make_in_maps(fmap1, fmap2, coords)
    res = run_bass_kernel_spmd(nc, in_maps, list(range(NCORES)))
    parts = [res.results[c]["out"] for c in range(NCORES)]  # [324, 8, 128] each
    full = np.concatenate(parts, axis=1)  # [324, 64, 128]
    return full[None].astype(np.float32)
